# revision 18
# baseline (speedup 1.0000x reference)
"""Single-head attention (nn_MultiHeadAttention) Trainium2 Bass kernel, v2.

Full inputs: x [4, 2048, 1024], Wq/Wk/Wv/Wo [1024, 1024], biases [1024].
reference:  q = x @ Wq.T + bq ; k,v likewise
            scores = (q @ k.T) / sqrt(1024) ; attn = softmax(scores, -1)
            out = (attn @ v) @ Wo.T + bo

Sharding: 8 cores = 4 batches x 2 query-halves; each core owns 1024
queries and all 2048 keys of its batch.

Algebraic fusion: scores = x (Wq^T Wk) x^T + (bq Wk) x^T + per-query
consts (which cancel in softmax).  A = Wq^T Wk is precomputed on the
host, so the K projection (and its DRAM spill) disappears: scores
contract directly against the resident x tiles.  The per-key offset
o_k = x_k . (bq Wk) rides in through the exp's per-partition bias.

Per-core pipeline (all matmul operands bf16, fp32 PSUM accumulation):
  QA phase:  QAT[d',q] = A^T x_q^T       (d-outer: PE starts after the
                                          first 0.25MB of A and x land)
  scores:    u[k,q]    = exp((QAT^T x)^T * scale + o_k * scale)
             Z[q]      = sum_k u        (vector-engine accumulation,
                                         cross-partition via ones-matmul)
  V phase:   V[s,e]    = x^T Wv^T + bv
  ctx:       ctxT[e,q] = V^T u
  out:       out[q,f]  = (ctxT^T Wo^T) * (1/Z) + bo
"""

import numpy as np
from contextlib import ExitStack

import ml_dtypes

import concourse.bass as bass
import concourse.bacc as bacc
import concourse.mybir as mybir
import concourse.tile as tile
from concourse import bass_utils

F32 = mybir.dt.float32
F32R = mybir.dt.float32r
BF16 = mybir.dt.bfloat16
AF = mybir.ActivationFunctionType
ALU = mybir.AluOpType

B, S, D = 4, 2048, 1024
SQ = S // 2  # queries per core
N_CORES = 8
SCALE = 1.0 / float(np.sqrt(D))

# matmul operand dtypes (PSUM accumulation is always fp32)
G1DT = BF16   # x, A, qa, wv  (QA / scores / V matmuls)
G2DT = BF16   # v, u          (ctx matmuls)
G3DT = BF16   # ctx, wo       (out-projection matmuls)


def build_nc():
    P = 128
    DT = D // P          # contraction tiles (8)
    ET = D // P          # output-dim tiles (8)
    SQW = 512            # query free-dim block
    SQB = SQ // SQW      # (2)
    SQT = SQ // P        # query tiles (8)
    SKT = S // P         # key tiles (16)
    NBW = 512            # free-dim block over D for V/out phases
    NB = D // NBW        # (2)

    nc = bacc.Bacc("TRN2", target_bir_lowering=False, debug=False)

    # all inputs pre-tiled on the host so every DMA chunk is one contiguous
    # DRAM run (strided row-chunks cap DMA throughput on descriptor overhead)
    xTt = nc.dram_tensor("xTt", [D // 128, 128, S], G1DT, kind="ExternalInput")
    aMt = nc.dram_tensor("aMt", [D // 128, 128, D], G1DT, kind="ExternalInput")
    wvTt = nc.dram_tensor("wvTt", [D // 128, 128, D], G1DT, kind="ExternalInput")
    woTt = nc.dram_tensor("woTt", [D // 128, 128, D], G3DT, kind="ExternalInput")
    bvd = nc.dram_tensor("bv", [D], F32, kind="ExternalInput")
    bod = nc.dram_tensor("bo", [D], F32, kind="ExternalInput")
    soffd = nc.dram_tensor("soff", [S], F32, kind="ExternalInput")
    outd = nc.dram_tensor("out", [SQ // 128, D // 512, 128, 512], F32,
                          kind="ExternalOutput")

    def bcast_ap(handle):
        a = handle[:]
        return bass.AP(tensor=a.tensor, offset=a.offset, ap=[[0, P]] + list(a.ap))

    with tile.TileContext(nc) as tc, ExitStack() as top:
        psum = top.enter_context(tc.tile_pool(name="psum", bufs=8, space="PSUM"))
        dram = top.enter_context(tc.tile_pool(name="dram", bufs=1, space="DRAM"))
        singles = top.enter_context(tc.tile_pool(name="singles", bufs=1))
        zscr = dram.tile([SQ], F32, name="zscr", tag="zscr")

        # ---- right-side pools, reserved in release order (LIFO top last)
        v_pool = tc.alloc_tile_pool(name="v", bufs=SKT, side="right")
        v_tiles = [v_pool.tile([P, D], G2DT, name=f"v{i}", tag="v")
                   for i in range(SKT)]
        u_pool = tc.alloc_tile_pool(name="u", bufs=SKT * SQB, side="right")
        u_tiles = [[None] * SKT for _ in range(SQB)]
        zacc_pool = tc.alloc_tile_pool(name="zacc", bufs=SQB, side="right")
        wv_pool = tc.alloc_tile_pool(name="wv", bufs=1, side="right")

        # ---- left-side: xt under qa under a_row (released in reverse)
        xt_pool = tc.alloc_tile_pool(name="xt", bufs=DT)
        qa_pool = tc.alloc_tile_pool(name="qa", bufs=ET)
        qa_tiles = [qa_pool.tile([P, SQ], G1DT, name=f"qa{i}", tag="qa")
                    for i in range(ET)]
        a_pool = tc.alloc_tile_pool(name="arow", bufs=DT)

        # QA-phase inputs round-robined over the three DMA-capable engine
        # queues in consumption order: the d-loop of sb=0 needs a_row[d] +
        # xt[d][:, 0:512]; the sb=1 pass needs xt[d][:, 512:1024]; the
        # scores phase then reads the key tail xt[d][:, 1024:2048].
        QE = [nc.sync, nc.gpsimd, nc.scalar]
        xt_tiles = []
        a_rows = []
        rr = 0
        for d in range(DT):
            ar = a_pool.tile([P, D], G1DT, name=f"ar{d}", tag="ar")
            xt_t = xt_pool.tile([P, S], G1DT, name=f"xt{d}", tag="xt")
            for h in range(2):
                QE[rr % 3].dma_start(
                    out=ar[:, h * SQW:(h + 1) * SQW],
                    in_=aMt[d, :, h * SQW:(h + 1) * SQW])
                rr += 1
            QE[rr % 3].dma_start(out=xt_t[:, 0:SQW], in_=xTt[d, :, 0:SQW])
            rr += 1
            a_rows.append(ar)
            xt_tiles.append(xt_t)
        for d in range(DT):
            QE[rr % 3].dma_start(out=xt_tiles[d][:, SQW:SQ], in_=xTt[d, :, SQW:SQ])
            rr += 1

        def xt_slice(d, lo, width):
            return xt_tiles[d][:, lo:lo + width]

        # key-half tail of x (needed from the scores phase on)
        for t in range(DT):
            QE[rr % 3].dma_start(out=xt_tiles[t][:, SQ:S], in_=xTt[t, :, SQ:S])
            rr += 1
        wv_full = wv_pool.tile([P, DT, D], G1DT, name="wv", tag="wv")
        for d in range(DT):
            nc.gpsimd.dma_start(out=wv_full[:, d, :], in_=wvTt[d])

        # constants / biases (emitted after the start-critical loads)
        ones_f32 = singles.tile([P, 1], F32, name="ones_f32", tag="ones_f32")
        nc.vector.memset(ones_f32, 1.0)
        ones_col = singles.tile([P, 1], F32R, name="ones_col", tag="ones_col")
        nc.scalar.activation(out=ones_col, in_=ones_f32, func=AF.Copy)
        soff_pt = singles.tile([P, SKT], F32, name="soff_pt", tag="soff_pt")
        nc.gpsimd.dma_start(out=soff_pt, in_=soffd[:].rearrange("(t p) -> p t", p=P))
        bv_bc = singles.tile([P, D], F32, name="bv_bc", tag="bv_bc")
        nc.gpsimd.dma_start(out=bv_bc, in_=bcast_ap(bvd))
        rzt = singles.tile([P, SQT], F32, name="rzt", tag="rzt")
        zt = singles.tile([P, SQT], F32, name="zt", tag="zt")
        z_sb = singles.tile([1, SQ], F32, name="z_sb", tag="z_sb")

        # ---------------- QA phase (d-outer for fast start) ----------------
        for sb in range(SQB):
            pq = [psum.tile([P, SQW], F32, name="mm", tag="mm") for _ in range(ET)]
            for d in range(DT):
                for et in range(ET):
                    nc.tensor.matmul(
                        pq[et],
                        lhsT=a_rows[d][:, et * P:(et + 1) * P],
                        rhs=xt_tiles[d][:, sb * SQW:(sb + 1) * SQW],
                        start=(d == 0), stop=(d == DT - 1),
                    )
            for et in range(ET):
                nc.scalar.activation(
                    out=qa_tiles[et][:, sb * SQW:(sb + 1) * SQW],
                    in_=pq[et], func=AF.Copy,
                )
        a_pool.release()

        # ---------------- scores + Z ----------------
        for sk in range(SKT):
            for q in range(SQB):
                ps = psum.tile([P, SQW], F32, name="mm", tag="mm")
                for e in range(ET):
                    nc.tensor.matmul(
                        ps,
                        lhsT=xt_slice(e, sk * P, P),
                        rhs=qa_tiles[e][:, q * SQW:(q + 1) * SQW],
                        start=(e == 0), stop=(e == ET - 1),
                    )
                ut = u_pool.tile([P, SQW], G2DT, name=f"u{q}_{sk}", tag="u")
                nc.scalar.activation(
                    out=ut, in_=ps, func=AF.Exp,
                    bias=soff_pt[:, sk:sk + 1], scale=SCALE,
                )
                u_tiles[q][sk] = ut
                if sk == 0:
                    za = zacc_pool.tile([P, SQW], F32R, name=f"za{q}", tag="za")
                    nc.vector.tensor_copy(za, ut)
                    if q == 0:
                        zacc = [za]
                    else:
                        zacc.append(za)
                else:
                    nc.vector.tensor_tensor(
                        out=zacc[q], in0=zacc[q], in1=ut, op=ALU.add)

        # ---------------- V phase ----------------
        for s in range(SKT):
            for eb in range(NB):
                pv = psum.tile([P, NBW], F32, name="mm", tag="mm")
                for d in range(DT):
                    nc.tensor.matmul(
                        pv,
                        lhsT=xt_slice(d, s * P, P),
                        rhs=wv_full[:, d, eb * NBW:(eb + 1) * NBW],
                        start=(d == 0), stop=(d == DT - 1),
                    )
                nc.vector.scalar_tensor_tensor(
                    out=v_tiles[s][:, eb * NBW:(eb + 1) * NBW],
                    in0=pv, scalar=1.0,
                    in1=bv_bc[:, eb * NBW:(eb + 1) * NBW],
                    op0=ALU.mult, op1=ALU.add,
                )
        # Z -> 1/Z in [q_p, st] layout (DRAM round-trip transpose); emitted
        # after the V matmuls so the z path never gates the PE stream
        for q in range(SQB):
            pz = psum.tile([1, SQW], F32, name="mm", tag="mm")
            nc.tensor.matmul(pz, lhsT=ones_col, rhs=zacc[q], start=True, stop=True)
            nc.scalar.copy(z_sb[0:1, q * SQW:(q + 1) * SQW], pz)
        nc.gpsimd.dma_start(out=zscr, in_=z_sb[0:1, :])
        nc.gpsimd.dma_start(out=zt, in_=zscr[:].rearrange("(t p) -> p t", p=P))
        nc.vector.reciprocal(out=rzt, in_=zt)

        wv_pool.release()
        zacc_pool.release()
        qa_pool.release()
        xt_pool.release()

        # ---------------- ctx phase (wo streams in behind it) ----------------
        ctx_pool = tc.alloc_tile_pool(name="ctx", bufs=ET)
        ctx_tiles = [ctx_pool.tile([P, SQ], G3DT, name=f"ctx{i}", tag="ctx")
                     for i in range(ET)]
        wo_pool = tc.alloc_tile_pool(name="wo", bufs=1)
        wo_full = wo_pool.tile([P, ET, D], G3DT, name="wo", tag="wo")
        for e in range(ET):
            nc.sync.dma_start(out=wo_full[:, e, :], in_=woTt[e])
        bo_bc = singles.tile([P, D], F32, name="bo_bc", tag="bo_bc")
        nc.gpsimd.dma_start(out=bo_bc, in_=bcast_ap(bod))

        for q in range(SQB):
            for e in range(ET):
                pc = psum.tile([P, SQW], F32, name="mm", tag="mm")
                for sk in range(SKT):
                    nc.tensor.matmul(
                        pc,
                        lhsT=v_tiles[sk][:, e * P:(e + 1) * P],
                        rhs=u_tiles[q][sk],
                        start=(sk == 0), stop=(sk == SKT - 1),
                    )
                nc.scalar.copy(ctx_tiles[e][:, q * SQW:(q + 1) * SQW], pc)
        u_pool.release()
        v_pool.release()

        # ---------------- out projection ----------------
        with tc.tile_pool(name="ofly", bufs=3) as o_pool:
            for st in range(SQT):
                for fb in range(NB):
                    po = psum.tile([P, NBW], F32, name="mm", tag="mm")
                    for e in range(ET):
                        nc.tensor.matmul(
                            po,
                            lhsT=ctx_tiles[e][:, st * P:(st + 1) * P],
                            rhs=wo_full[:, e, fb * NBW:(fb + 1) * NBW],
                            start=(e == 0), stop=(e == ET - 1),
                        )
                    osb = o_pool.tile([P, NBW], F32, name="osb", tag="ofly")
                    nc.vector.scalar_tensor_tensor(
                        out=osb, in0=po, scalar=rzt[:, st:st + 1],
                        in1=bo_bc[:, fb * NBW:(fb + 1) * NBW],
                        op0=ALU.mult, op1=ALU.add,
                    )
                    nc.scalar.dma_start(out=outd[st, fb], in_=osb)
        wo_pool.release()
        ctx_pool.release()

    nc.compile()
    return nc


_NC_CACHE = {}


def _get_nc():
    if "nc" not in _NC_CACHE:
        _NC_CACHE["nc"] = build_nc()
    return _NC_CACHE["nc"]


def _round_f32r(a):
    """Round-to-nearest to fp32r precision (fp22 = s1e8m13)."""
    u = np.ascontiguousarray(a, np.float32).view(np.uint32)
    u = ((u.astype(np.uint64) + 0x200) & 0xFFFFFC00).astype(np.uint32)
    return u.view(np.float32)


def _cast(a, dt):
    a = np.ascontiguousarray(np.asarray(a, np.float32))
    if dt == BF16:
        return a.astype(ml_dtypes.bfloat16)
    if dt == F32R:
        return _round_f32r(a)
    return a


def _tile_rows(m, dt):
    """[D, N] -> contiguous [D//128, 128, N] row-tiles, cast to dt."""
    m = np.asarray(m, np.float32)
    return np.ascontiguousarray(_cast(m, dt).reshape(m.shape[0] // 128, 128, -1))


def make_in_maps(x, Wq, bq, Wk, bk, Wv, bv, Wo, bo):
    x = np.asarray(x, np.float32)
    Wq = np.asarray(Wq, np.float32)
    Wk = np.asarray(Wk, np.float32)
    # A = Wq^T Wk so scores = x A x^T (+ per-key offset from bq, see header)
    aMt = _tile_rows(Wq.T @ Wk, G1DT)
    wvTt = _tile_rows(np.asarray(Wv, np.float32).T, G1DT)
    woTt = _tile_rows(np.asarray(Wo, np.float32).T, G3DT)
    bv = np.ascontiguousarray(np.asarray(bv, np.float32))
    bo = np.ascontiguousarray(np.asarray(bo, np.float32))
    ck = np.asarray(bq, np.float32) @ Wk  # [d]

    in_maps = []
    for c in range(N_CORES):
        b, h = c // 2, c % 2
        xb = x[b]  # [S, D]
        mine = xb[h * SQ:(h + 1) * SQ]
        other = xb[(1 - h) * SQ:(2 - h) * SQ]
        perm = np.concatenate([mine, other], axis=0)  # [S, D], own queries first
        xTt = _tile_rows(perm.T, G1DT)
        soff = np.ascontiguousarray((perm @ ck) * np.float32(SCALE))
        in_maps.append({
            "xTt": xTt, "aMt": aMt, "wvTt": wvTt, "woTt": woTt,
            "bv": bv, "bo": bo, "soff": soff,
        })
    return in_maps


def assemble(results):
    out = np.empty((B, S, D), np.float32)
    for c in range(N_CORES):
        b, h = c // 2, c % 2
        # [8, 2, 128, 512] tiled -> [1024, 1024]
        blk = np.asarray(results[c]["out"])
        out[b, h * SQ:(h + 1) * SQ] = (
            blk.transpose(0, 2, 1, 3).reshape(SQ, D))
    return out


def kernel(x, Wq, bq, Wk, bk, Wv, bv, Wo, bo, **kwargs):
    nc = _get_nc()
    in_maps = make_in_maps(x, Wq, bq, Wk, bk, Wv, bv, Wo, bo)
    res = bass_utils.run_bass_kernel_spmd(nc, in_maps, core_ids=list(range(N_CORES)))
    return assemble(res.results)


# revision 20
# speedup vs baseline: 1.0013x; 1.0013x over previous
"""Single-head attention (nn_MultiHeadAttention) Trainium2 Bass kernel, v5.

Full inputs: x [4, 2048, 1024], Wq/Wk/Wv/Wo [1024, 1024], biases [1024].
reference:  q = x @ Wq.T + bq ; k,v likewise
            scores = (q @ k.T) / sqrt(1024) ; attn = softmax(scores, -1)
            out = (attn @ v) @ Wo.T + bo

Sharding: 8 cores = 4 batches x 2 query-halves; each core owns 1024
queries and all 2048 keys of its batch (global key order everywhere).

Algebraic fusion: scores = x (Wq^T Wk) x^T + (bq Wk) x^T + per-query
consts (which cancel in softmax).  A = Wq^T Wk is precomputed on the
host, so the K projection (and its DRAM spill) disappears: scores
contract directly against the resident x tiles.  The per-key offset
o_k = x_k . (bq Wk) rides in through the exp's per-partition bias.

V dedup: each core projects V only for its OWN 1024 keys (which equal
its own query rows, passed as the separate xq input so the program
stays SPMD-uniform), then the core pair exchanges halves with a
pairwise AllGather through a DRAM bounce, hidden behind the scores
phase.

Per-core pipeline (all matmul operands bf16, fp32 PSUM accumulation):
  QA phase:  QAT[d',q]  = A^T xq^T          (d-outer for fast start)
  V phase:   Vown[s,e]  = xq^T Wv^T + bv    (own 1024 keys only)
             spill -> AllGather[pair] -> reload full V   (async)
  scores:    u[k,q]     = exp((QAT^T x)^T * scale + o_k * scale)
             Z[q]       = sum_k u           (vector-engine accumulation)
  ctx:       ctxT[e,q]  = V^T u
  out:       out[q,f]   = (ctxT^T Wo^T) * (1/Z) + bo
"""

import numpy as np
from contextlib import ExitStack

import ml_dtypes

import concourse.bass as bass
import concourse.bacc as bacc
import concourse.mybir as mybir
import concourse.tile as tile
from concourse import bass_utils

F32 = mybir.dt.float32
F32R = mybir.dt.float32r
BF16 = mybir.dt.bfloat16
AF = mybir.ActivationFunctionType
ALU = mybir.AluOpType

B, S, D = 4, 2048, 1024
SQ = S // 2  # queries per core
N_CORES = 8
SCALE = 1.0 / float(np.sqrt(D))

# matmul operand dtypes (PSUM accumulation is always fp32)
G1DT = BF16   # x, A, qa, wv  (QA / scores / V matmuls)
G2DT = BF16   # v, u          (ctx matmuls)
G3DT = BF16   # ctx, wo       (out-projection matmuls)


def build_nc():
    P = 128
    DT = D // P          # contraction tiles (8)
    ET = D // P          # output-dim tiles (8)
    SQW = 512            # query free-dim block
    SQB = SQ // SQW      # (2)
    SQT = SQ // P        # query tiles (8)
    SKT = S // P         # key tiles (16)
    SOT = SQ // P        # own-key tiles (8)
    NBW = 512            # free-dim block over D for V/out phases
    NB = D // NBW        # (2)

    nc = bacc.Bacc("TRN2", target_bir_lowering=False, debug=False,
                   num_devices=N_CORES)

    # all inputs pre-tiled on the host so every DMA chunk is one contiguous
    # DRAM run (strided row-chunks cap DMA throughput on descriptor overhead)
    xTt = nc.dram_tensor("xTt", [DT, P, S], G1DT, kind="ExternalInput")
    xqt = nc.dram_tensor("xqt", [DT, P, SQ], G1DT, kind="ExternalInput")
    aMt = nc.dram_tensor("aMt", [DT, P, D], G1DT, kind="ExternalInput")
    wvTt = nc.dram_tensor("wvTt", [DT, P, D], G1DT, kind="ExternalInput")
    woTt = nc.dram_tensor("woTt", [ET, P, D], G3DT, kind="ExternalInput")
    bvd = nc.dram_tensor("bv", [D], F32, kind="ExternalInput")
    bod = nc.dram_tensor("bo", [D], F32, kind="ExternalInput")
    soffd = nc.dram_tensor("soff", [S], F32, kind="ExternalInput")
    outd = nc.dram_tensor("out", [SQT, NB, P, NBW], F32, kind="ExternalOutput")

    def bcast_ap(handle):
        a = handle[:]
        return bass.AP(tensor=a.tensor, offset=a.offset, ap=[[0, P]] + list(a.ap))

    with tile.TileContext(nc) as tc, ExitStack() as top:
        psum = top.enter_context(tc.tile_pool(name="psum", bufs=8, space="PSUM"))
        dram = top.enter_context(tc.tile_pool(name="dram", bufs=1, space="DRAM"))
        singles = top.enter_context(tc.tile_pool(name="singles", bufs=1))
        zscr = dram.tile([SQ], F32, name="zscr", tag="zscr")
        vb_in = dram.tile([SOT, P, D], G2DT, name="vb_in", tag="vb_in")
        vb_out = dram.tile([2, SOT, P, D], G2DT, name="vb_out", tag="vb_out")

        # ---- right-side pools, reserved in release order (LIFO top last)
        v_pool = tc.alloc_tile_pool(name="v", bufs=SKT, side="right")
        v_tiles = [v_pool.tile([P, D], G2DT, name=f"v{i}", tag="v")
                   for i in range(SKT)]
        u_pool = tc.alloc_tile_pool(name="u", bufs=SKT * SQB, side="right")
        u_tiles = [[None] * SKT for _ in range(SQB)]
        zacc_pool = tc.alloc_tile_pool(name="zacc", bufs=SQB, side="right")
        wv_pool = tc.alloc_tile_pool(name="wv", bufs=1, side="right")
        vown_pool = tc.alloc_tile_pool(name="vown", bufs=SOT, side="right")

        # ---- left-side: xt/xq under qa under a_row (released in reverse)
        xt_pool = tc.alloc_tile_pool(name="xt", bufs=DT)
        xq_pool = tc.alloc_tile_pool(name="xq", bufs=DT)
        qa_pool = tc.alloc_tile_pool(name="qa", bufs=ET)
        qa_tiles = [qa_pool.tile([P, SQ], G1DT, name=f"qa{i}", tag="qa")
                    for i in range(ET)]
        a_pool = tc.alloc_tile_pool(name="arow", bufs=DT)

        # startup-critical loads round-robined over the three DMA-capable
        # engine queues in consumption order: the QA d-loop of sb=0 needs
        # a_row[d] + xq[d][:, 0:512]; sb=1 needs xq[d][:, 512:1024]; the V
        # phase then reads xq again; scores reads the full global xt.
        QE = [nc.sync, nc.gpsimd, nc.scalar]
        xt_tiles = []
        xq_tiles = []
        a_rows = []
        rr = 0
        for d in range(DT):
            ar = a_pool.tile([P, D], G1DT, name=f"ar{d}", tag="ar")
            xq_t = xq_pool.tile([P, SQ], G1DT, name=f"xq{d}", tag="xq")
            for h in range(2):
                QE[rr % 3].dma_start(
                    out=ar[:, h * SQW:(h + 1) * SQW],
                    in_=aMt[d, :, h * SQW:(h + 1) * SQW])
                rr += 1
            QE[rr % 3].dma_start(out=xq_t[:, 0:SQW], in_=xqt[d, :, 0:SQW])
            rr += 1
            a_rows.append(ar)
            xq_tiles.append(xq_t)
        for d in range(DT):
            QE[rr % 3].dma_start(out=xq_tiles[d][:, SQW:SQ], in_=xqt[d, :, SQW:SQ])
            rr += 1

        # wv next (V phase), then the global x tiles (scores phase)
        wv_full = wv_pool.tile([P, DT, D], G1DT, name="wv", tag="wv")
        for d in range(DT):
            nc.gpsimd.dma_start(out=wv_full[:, d, :], in_=wvTt[d])
        for t in range(DT):
            xt_t = xt_pool.tile([P, S], G1DT, name=f"xt{t}", tag="xt")
            QE[rr % 3].dma_start(out=xt_t[:, 0:SQ], in_=xTt[t, :, 0:SQ])
            rr += 1
            QE[rr % 3].dma_start(out=xt_t[:, SQ:S], in_=xTt[t, :, SQ:S])
            rr += 1
            xt_tiles.append(xt_t)

        def xt_slice(d, lo, width):
            return xt_tiles[d][:, lo:lo + width]

        # constants / biases (emitted after the start-critical loads)
        ones_f32 = singles.tile([P, 1], F32, name="ones_f32", tag="ones_f32")
        nc.vector.memset(ones_f32, 1.0)
        ones_col = singles.tile([P, 1], F32R, name="ones_col", tag="ones_col")
        nc.scalar.activation(out=ones_col, in_=ones_f32, func=AF.Copy)
        soff_pt = singles.tile([P, SKT], F32, name="soff_pt", tag="soff_pt")
        nc.gpsimd.dma_start(out=soff_pt, in_=soffd[:].rearrange("(t p) -> p t", p=P))
        bv_bc = singles.tile([P, D], F32, name="bv_bc", tag="bv_bc")
        nc.gpsimd.dma_start(out=bv_bc, in_=bcast_ap(bvd))
        rzt = singles.tile([P, SQT], F32, name="rzt", tag="rzt")
        zt = singles.tile([P, SQT], F32, name="zt", tag="zt")
        z_sb = singles.tile([1, SQ], F32, name="z_sb", tag="z_sb")

        # ---------------- QA phase (d-outer for fast start) ----------------
        for sb in range(SQB):
            pq = [psum.tile([P, SQW], F32, name="mm", tag="mm") for _ in range(ET)]
            for d in range(DT):
                for et in range(ET):
                    nc.tensor.matmul(
                        pq[et],
                        lhsT=a_rows[d][:, et * P:(et + 1) * P],
                        rhs=xq_tiles[d][:, sb * SQW:(sb + 1) * SQW],
                        start=(d == 0), stop=(d == DT - 1),
                    )
            for et in range(ET):
                nc.scalar.activation(
                    out=qa_tiles[et][:, sb * SQW:(sb + 1) * SQW],
                    in_=pq[et], func=AF.Copy,
                )
        a_pool.release()

        # ---------------- V phase (own keys only) ----------------
        vown_tiles = [vown_pool.tile([P, D], G2DT, name=f"vo{i}", tag="vo")
                      for i in range(SOT)]
        for s in range(SOT):
            for eb in range(NB):
                pv = psum.tile([P, NBW], F32, name="mm", tag="mm")
                for d in range(DT):
                    nc.tensor.matmul(
                        pv,
                        lhsT=xq_tiles[d][:, s * P:(s + 1) * P],
                        rhs=wv_full[:, d, eb * NBW:(eb + 1) * NBW],
                        start=(d == 0), stop=(d == DT - 1),
                    )
                nc.vector.scalar_tensor_tensor(
                    out=vown_tiles[s][:, eb * NBW:(eb + 1) * NBW],
                    in0=pv, scalar=1.0,
                    in1=bv_bc[:, eb * NBW:(eb + 1) * NBW],
                    op0=ALU.mult, op1=ALU.add,
                )
        # pairwise exchange: spill own half, AllGather, reload both halves.
        # Runs behind the scores phase; Tile tracks the DRAM deps.
        for s in range(SOT):
            nc.sync.dma_start(out=vb_in[s], in_=vown_tiles[s])
        nc.gpsimd.collective_compute(
            "AllGather",
            ALU.bypass,
            replica_groups=[[0, 1], [2, 3], [4, 5], [6, 7]],
            ins=[vb_in[:]],
            outs=[vb_out[:]],
        )
        for sk in range(SKT):
            nc.sync.dma_start(out=v_tiles[sk], in_=vb_out[sk // SOT, sk % SOT])

        # ---------------- scores + Z ----------------
        for sk in range(SKT):
            for q in range(SQB):
                ps = psum.tile([P, SQW], F32, name="mm", tag="mm")
                for e in range(ET):
                    nc.tensor.matmul(
                        ps,
                        lhsT=xt_slice(e, sk * P, P),
                        rhs=qa_tiles[e][:, q * SQW:(q + 1) * SQW],
                        start=(e == 0), stop=(e == ET - 1),
                    )
                ut = u_pool.tile([P, SQW], G2DT, name=f"u{q}_{sk}", tag="u")
                nc.scalar.activation(
                    out=ut, in_=ps, func=AF.Exp,
                    bias=soff_pt[:, sk:sk + 1], scale=SCALE,
                )
                u_tiles[q][sk] = ut
                if sk == 0:
                    za = zacc_pool.tile([P, SQW], F32R, name=f"za{q}", tag="za")
                    nc.vector.tensor_copy(za, ut)
                    if q == 0:
                        zacc = [za]
                    else:
                        zacc.append(za)
                else:
                    nc.vector.tensor_tensor(
                        out=zacc[q], in0=zacc[q], in1=ut, op=ALU.add)

        # Z -> 1/Z in [q_p, st] layout (DRAM round-trip transpose)
        for q in range(SQB):
            pz = psum.tile([1, SQW], F32, name="mm", tag="mm")
            nc.tensor.matmul(pz, lhsT=ones_col, rhs=zacc[q], start=True, stop=True)
            nc.scalar.copy(z_sb[0:1, q * SQW:(q + 1) * SQW], pz)
        nc.gpsimd.dma_start(out=zscr, in_=z_sb[0:1, :])
        nc.gpsimd.dma_start(out=zt, in_=zscr[:].rearrange("(t p) -> p t", p=P))
        nc.vector.reciprocal(out=rzt, in_=zt)

        vown_pool.release()
        wv_pool.release()
        zacc_pool.release()
        qa_pool.release()
        xq_pool.release()
        xt_pool.release()

        # ---------------- ctx phase (wo streams in behind it) ----------------
        ctx_pool = tc.alloc_tile_pool(name="ctx", bufs=ET)
        ctx_tiles = [ctx_pool.tile([P, SQ], G3DT, name=f"ctx{i}", tag="ctx")
                     for i in range(ET)]
        wo_pool = tc.alloc_tile_pool(name="wo", bufs=1)
        wo_full = wo_pool.tile([P, ET, D], G3DT, name="wo", tag="wo")
        for e in range(ET):
            nc.sync.dma_start(out=wo_full[:, e, :], in_=woTt[e])
        bo_bc = singles.tile([P, D], F32, name="bo_bc", tag="bo_bc")
        nc.gpsimd.dma_start(out=bo_bc, in_=bcast_ap(bod))

        for q in range(SQB):
            for e in range(ET):
                pc = psum.tile([P, SQW], F32, name="mm", tag="mm")
                for sk in range(SKT):
                    nc.tensor.matmul(
                        pc,
                        lhsT=v_tiles[sk][:, e * P:(e + 1) * P],
                        rhs=u_tiles[q][sk],
                        start=(sk == 0), stop=(sk == SKT - 1),
                    )
                nc.scalar.copy(ctx_tiles[e][:, q * SQW:(q + 1) * SQW], pc)
        u_pool.release()
        v_pool.release()

        # ---------------- out projection ----------------
        with tc.tile_pool(name="ofly", bufs=3) as o_pool:
            for st in range(SQT):
                for fb in range(NB):
                    po = psum.tile([P, NBW], F32, name="mm", tag="mm")
                    for e in range(ET):
                        nc.tensor.matmul(
                            po,
                            lhsT=ctx_tiles[e][:, st * P:(st + 1) * P],
                            rhs=wo_full[:, e, fb * NBW:(fb + 1) * NBW],
                            start=(e == 0), stop=(e == ET - 1),
                        )
                    osb = o_pool.tile([P, NBW], F32, name="osb", tag="ofly")
                    nc.vector.scalar_tensor_tensor(
                        out=osb, in0=po, scalar=rzt[:, st:st + 1],
                        in1=bo_bc[:, fb * NBW:(fb + 1) * NBW],
                        op0=ALU.mult, op1=ALU.add,
                    )
                    nc.scalar.dma_start(out=outd[st, fb], in_=osb)
        wo_pool.release()
        ctx_pool.release()

    nc.compile()
    return nc


_NC_CACHE = {}


def _get_nc():
    if "nc" not in _NC_CACHE:
        _NC_CACHE["nc"] = build_nc()
    return _NC_CACHE["nc"]


def _round_f32r(a):
    """Round-to-nearest to fp32r precision (fp22 = s1e8m13)."""
    u = np.ascontiguousarray(a, np.float32).view(np.uint32)
    u = ((u.astype(np.uint64) + 0x200) & 0xFFFFFC00).astype(np.uint32)
    return u.view(np.float32)


def _cast(a, dt):
    a = np.ascontiguousarray(np.asarray(a, np.float32))
    if dt == BF16:
        return a.astype(ml_dtypes.bfloat16)
    if dt == F32R:
        return _round_f32r(a)
    return a


def _tile_rows(m, dt):
    """[D, N] -> contiguous [D//128, 128, N] row-tiles, cast to dt."""
    m = np.asarray(m, np.float32)
    return np.ascontiguousarray(_cast(m, dt).reshape(m.shape[0] // 128, 128, -1))


def make_in_maps(x, Wq, bq, Wk, bk, Wv, bv, Wo, bo):
    x = np.asarray(x, np.float32)
    Wq = np.asarray(Wq, np.float32)
    Wk = np.asarray(Wk, np.float32)
    # A = Wq^T Wk so scores = x A x^T (+ per-key offset from bq, see header)
    aMt = _tile_rows(Wq.T @ Wk, G1DT)
    wvTt = _tile_rows(np.asarray(Wv, np.float32).T, G1DT)
    woTt = _tile_rows(np.asarray(Wo, np.float32).T, G3DT)
    bv = np.ascontiguousarray(np.asarray(bv, np.float32))
    bo = np.ascontiguousarray(np.asarray(bo, np.float32))
    ck = np.asarray(bq, np.float32) @ Wk  # [d]

    in_maps = []
    for c in range(N_CORES):
        b, h = c // 2, c % 2
        xb = x[b]  # [S, D], global key order
        own = xb[h * SQ:(h + 1) * SQ]
        xTt_c = _tile_rows(xb.T, G1DT)
        xqt_c = _tile_rows(own.T, G1DT)
        soff = np.ascontiguousarray((xb @ ck) * np.float32(SCALE))
        in_maps.append({
            "xTt": xTt_c, "xqt": xqt_c, "aMt": aMt, "wvTt": wvTt, "woTt": woTt,
            "bv": bv, "bo": bo, "soff": soff,
        })
    return in_maps


def assemble(results):
    out = np.empty((B, S, D), np.float32)
    for c in range(N_CORES):
        b, h = c // 2, c % 2
        # [8, 2, 128, 512] tiled -> [1024, 1024]
        blk = np.asarray(results[c]["out"])
        out[b, h * SQ:(h + 1) * SQ] = (
            blk.transpose(0, 2, 1, 3).reshape(SQ, D))
    return out


def kernel(x, Wq, bq, Wk, bk, Wv, bv, Wo, bo, **kwargs):
    nc = _get_nc()
    in_maps = make_in_maps(x, Wq, bq, Wk, bk, Wv, bv, Wo, bo)
    res = bass_utils.run_bass_kernel_spmd(nc, in_maps, core_ids=list(range(N_CORES)))
    return assemble(res.results)


# revision 23
# speedup vs baseline: 1.1163x; 1.1149x over previous
"""Single-head attention (nn_MultiHeadAttention) Trainium2 Bass kernel, v5.

Full inputs: x [4, 2048, 1024], Wq/Wk/Wv/Wo [1024, 1024], biases [1024].
reference:  q = x @ Wq.T + bq ; k,v likewise
            scores = (q @ k.T) / sqrt(1024) ; attn = softmax(scores, -1)
            out = (attn @ v) @ Wo.T + bo

Sharding: 8 cores = 4 batches x 2 query-halves; each core owns 1024
queries and all 2048 keys of its batch (global key order everywhere).

Algebraic fusion: scores = x (Wq^T Wk) x^T + (bq Wk) x^T + per-query
consts (which cancel in softmax).  A = Wq^T Wk is precomputed on the
host, so the K projection (and its DRAM spill) disappears: scores
contract directly against the resident x tiles.  The per-key offset
o_k = x_k . (bq Wk) rides in through the exp's per-partition bias.

V dedup: each core projects V only for its OWN 1024 keys (which equal
its own query rows, passed as the separate xq input so the program
stays SPMD-uniform), then the core pair exchanges halves with a
pairwise AllGather through a DRAM bounce, hidden behind the scores
phase.

Per-core pipeline (all matmul operands bf16, fp32 PSUM accumulation):
  QA phase:  QAT[d',q]  = A^T xq^T          (d-outer for fast start)
  V phase:   Vown[s,e]  = xq^T Wv^T + bv    (own 1024 keys only)
             spill -> AllGather[pair] -> reload full V   (async)
  scores:    u[k,q]     = exp((QAT^T x)^T * scale + o_k * scale)
             Z[q]       = sum_k u           (vector-engine accumulation)
  ctx:       ctxT[e,q]  = V^T u
  out:       out[q,f]   = (ctxT^T Wo^T) * (1/Z) + bo
"""

import numpy as np
from contextlib import ExitStack

import ml_dtypes

import concourse.bass as bass
import concourse.bacc as bacc
import concourse.mybir as mybir
import concourse.tile as tile
from concourse import bass_utils

F32 = mybir.dt.float32
F32R = mybir.dt.float32r
BF16 = mybir.dt.bfloat16
AF = mybir.ActivationFunctionType
ALU = mybir.AluOpType

B, S, D = 4, 2048, 1024
SQ = S // 2  # queries per core
N_CORES = 8
SCALE = 1.0 / float(np.sqrt(D))

# matmul operand dtypes (PSUM accumulation is always fp32)
G1DT = BF16   # x, A, qa, wv  (QA / scores / V matmuls)
G2DT = BF16   # v, u          (ctx matmuls)
G3DT = BF16   # ctx, wo       (out-projection matmuls)


def build_nc():
    P = 128
    DT = D // P          # contraction tiles (8)
    ET = D // P          # output-dim tiles (8)
    SQW = 512            # query free-dim block
    SQB = SQ // SQW      # (2)
    SQT = SQ // P        # query tiles (8)
    SKT = S // P         # key tiles (16)
    SOT = SQ // P        # own-key tiles (8)
    NBW = 512            # free-dim block over D for V/out phases
    NB = D // NBW        # (2)

    nc = bacc.Bacc("TRN2", target_bir_lowering=False, debug=False,
                   num_devices=N_CORES)

    # all inputs pre-tiled on the host so every DMA chunk is one contiguous
    # DRAM run (strided row-chunks cap DMA throughput on descriptor overhead)
    xTt = nc.dram_tensor("xTt", [DT, P, S], G1DT, kind="ExternalInput")
    xqt = nc.dram_tensor("xqt", [DT, P, SQ], G1DT, kind="ExternalInput")
    aMt = nc.dram_tensor("aMt", [DT, P, D], G1DT, kind="ExternalInput")
    wvTt = nc.dram_tensor("wvTt", [DT, P, D], G1DT, kind="ExternalInput")
    woTt = nc.dram_tensor("woTt", [ET, P, D], G3DT, kind="ExternalInput")
    bvd = nc.dram_tensor("bv", [D], F32, kind="ExternalInput")
    bod = nc.dram_tensor("bo", [D], F32, kind="ExternalInput")
    soffd = nc.dram_tensor("soff", [S], F32, kind="ExternalInput")
    outd = nc.dram_tensor("out", [SQT, NB, P, NBW], F32, kind="ExternalOutput")

    def bcast_ap(handle):
        a = handle[:]
        return bass.AP(tensor=a.tensor, offset=a.offset, ap=[[0, P]] + list(a.ap))

    with tile.TileContext(nc) as tc, ExitStack() as top:
        psum = top.enter_context(tc.tile_pool(name="psum", bufs=8, space="PSUM"))
        dram = top.enter_context(tc.tile_pool(name="dram", bufs=1, space="DRAM"))
        singles = top.enter_context(tc.tile_pool(name="singles", bufs=1))
        zscr = dram.tile([SQ], F32, name="zscr", tag="zscr")
        vb_in = dram.tile([SOT, P, D], G2DT, name="vb_in", tag="vb_in")
        vb_out = dram.tile([2, SOT, P, D], G2DT, name="vb_out", tag="vb_out")

        # ---- right-side pools, reserved in release order (LIFO top last)
        v_pool = tc.alloc_tile_pool(name="v", bufs=SKT, side="right")
        v_tiles = [v_pool.tile([P, D], G2DT, name=f"v{i}", tag="v")
                   for i in range(SKT)]
        u_pool = tc.alloc_tile_pool(name="u", bufs=SKT * SQB, side="right")
        u_tiles = [[None] * SKT for _ in range(SQB)]
        zacc_pool = tc.alloc_tile_pool(name="zacc", bufs=SQB, side="right")
        wv_pool = tc.alloc_tile_pool(name="wv", bufs=1, side="right")
        vown_pool = tc.alloc_tile_pool(name="vown", bufs=SOT, side="right")

        # ---- left-side: xt/xq under qa under a_row (released in reverse)
        xt_pool = tc.alloc_tile_pool(name="xt", bufs=DT)
        xq_pool = tc.alloc_tile_pool(name="xq", bufs=DT)
        qa_pool = tc.alloc_tile_pool(name="qa", bufs=ET)
        qa_tiles = [qa_pool.tile([P, SQ], G1DT, name=f"qa{i}", tag="qa")
                    for i in range(ET)]
        a_pool = tc.alloc_tile_pool(name="arow", bufs=DT)

        # DMA plan: scalar stays free for ACT work (PSUM evacuation); sync
        # carries xq -> xt -> V spills -> V reloads -> wo; gpsimd carries
        # biases -> wv -> a -> the AllGather -> z round-trip.  Loads are
        # emitted in consumption order: the V d-loop of sgroup 0 needs
        # wv[d] + xq[d][:, 0:512] per d-step.
        bv_bc = singles.tile([P, D], F32, name="bv_bc", tag="bv_bc")
        nc.gpsimd.dma_start(out=bv_bc, in_=bcast_ap(bvd))
        wv_full = wv_pool.tile([P, DT, D], G1DT, name="wv", tag="wv")
        xq_tiles = []
        for d in range(DT):
            xq_t = xq_pool.tile([P, SQ], G1DT, name=f"xq{d}", tag="xq")
            nc.sync.dma_start(out=xq_t, in_=xqt[d])
            nc.gpsimd.dma_start(out=wv_full[:, d, :], in_=wvTt[d])
            xq_tiles.append(xq_t)
        a_rows = []
        for d in range(DT):
            ar = a_pool.tile([P, D], G1DT, name=f"ar{d}", tag="ar")
            nc.gpsimd.dma_start(out=ar, in_=aMt[d])
            a_rows.append(ar)
        xt_tiles = []
        for t in range(DT):
            xt_t = xt_pool.tile([P, S], G1DT, name=f"xt{t}", tag="xt")
            nc.sync.dma_start(out=xt_t, in_=xTt[t])
            xt_tiles.append(xt_t)

        def xt_slice(d, lo, width):
            return xt_tiles[d][:, lo:lo + width]

        # constants / biases (emitted after the start-critical loads)
        ones_f32 = singles.tile([P, 1], F32, name="ones_f32", tag="ones_f32")
        nc.vector.memset(ones_f32, 1.0)
        ones_col = singles.tile([P, 1], F32R, name="ones_col", tag="ones_col")
        nc.scalar.activation(out=ones_col, in_=ones_f32, func=AF.Copy)
        soff_pt = singles.tile([P, SKT], F32, name="soff_pt", tag="soff_pt")
        nc.gpsimd.dma_start(out=soff_pt, in_=soffd[:].rearrange("(t p) -> p t", p=P))
        rzt = singles.tile([P, SQT], F32, name="rzt", tag="rzt")
        zt = singles.tile([P, SQT], F32, name="zt", tag="zt")
        z_sb = singles.tile([1, SQ], F32, name="z_sb", tag="z_sb")

        # ---------------- V phase first (own keys only, d-outer) ----------
        # so the pair exchange starts early and hides behind QA + scores
        vown_tiles = [vown_pool.tile([P, D], G2DT, name=f"vo{i}", tag="vo")
                      for i in range(SOT)]
        for sg in range(2):
            pv = [psum.tile([P, NBW], F32, name="mm", tag="mm") for _ in range(8)]
            for d in range(DT):
                for si in range(4):
                    for eb in range(NB):
                        nc.tensor.matmul(
                            pv[si * 2 + eb],
                            lhsT=xq_tiles[d][:, (sg * 4 + si) * P:(sg * 4 + si + 1) * P],
                            rhs=wv_full[:, d, eb * NBW:(eb + 1) * NBW],
                            start=(d == 0), stop=(d == DT - 1),
                        )
            for si in range(4):
                for eb in range(NB):
                    s = sg * 4 + si
                    nc.vector.scalar_tensor_tensor(
                        out=vown_tiles[s][:, eb * NBW:(eb + 1) * NBW],
                        in0=pv[si * 2 + eb], scalar=1.0,
                        in1=bv_bc[:, eb * NBW:(eb + 1) * NBW],
                        op0=ALU.mult, op1=ALU.add,
                    )
            for si in range(4):
                s = sg * 4 + si
                nc.sync.dma_start(out=vb_in[s], in_=vown_tiles[s])
        # pairwise exchange: AllGather the spilled halves, reload both.
        nc.gpsimd.collective_compute(
            "AllGather",
            ALU.bypass,
            replica_groups=[[0, 1], [2, 3], [4, 5], [6, 7]],
            ins=[vb_in[:]],
            outs=[vb_out[:]],
        )
        for sk in range(SKT):
            nc.gpsimd.dma_start(out=v_tiles[sk], in_=vb_out[sk // SOT, sk % SOT])

        # ---------------- QA phase (d-outer) ----------------
        for sb in range(SQB):
            pq = [psum.tile([P, SQW], F32, name="mm", tag="mm") for _ in range(ET)]
            for d in range(DT):
                for et in range(ET):
                    nc.tensor.matmul(
                        pq[et],
                        lhsT=a_rows[d][:, et * P:(et + 1) * P],
                        rhs=xq_tiles[d][:, sb * SQW:(sb + 1) * SQW],
                        start=(d == 0), stop=(d == DT - 1),
                    )
            for et in range(ET):
                nc.scalar.activation(
                    out=qa_tiles[et][:, sb * SQW:(sb + 1) * SQW],
                    in_=pq[et], func=AF.Copy,
                )
        a_pool.release()

        # ---------------- scores + Z ----------------
        for sk in range(SKT):
            for q in range(SQB):
                ps = psum.tile([P, SQW], F32, name="mm", tag="mm")
                for e in range(ET):
                    nc.tensor.matmul(
                        ps,
                        lhsT=xt_slice(e, sk * P, P),
                        rhs=qa_tiles[e][:, q * SQW:(q + 1) * SQW],
                        start=(e == 0), stop=(e == ET - 1),
                    )
                ut = u_pool.tile([P, SQW], G2DT, name=f"u{q}_{sk}", tag="u")
                nc.scalar.activation(
                    out=ut, in_=ps, func=AF.Exp,
                    bias=soff_pt[:, sk:sk + 1], scale=SCALE,
                )
                u_tiles[q][sk] = ut
                if sk == 0:
                    za = zacc_pool.tile([P, SQW], F32R, name=f"za{q}", tag="za")
                    nc.vector.tensor_copy(za, ut)
                    if q == 0:
                        zacc = [za]
                    else:
                        zacc.append(za)
                else:
                    nc.vector.tensor_tensor(
                        out=zacc[q], in0=zacc[q], in1=ut, op=ALU.add)

        # Z -> 1/Z in [q_p, st] layout (DRAM round-trip transpose)
        for q in range(SQB):
            pz = psum.tile([1, SQW], F32, name="mm", tag="mm")
            nc.tensor.matmul(pz, lhsT=ones_col, rhs=zacc[q], start=True, stop=True)
            nc.scalar.copy(z_sb[0:1, q * SQW:(q + 1) * SQW], pz)
        nc.gpsimd.dma_start(out=zscr, in_=z_sb[0:1, :])
        nc.gpsimd.dma_start(out=zt, in_=zscr[:].rearrange("(t p) -> p t", p=P))
        nc.vector.reciprocal(out=rzt, in_=zt)

        vown_pool.release()
        wv_pool.release()
        zacc_pool.release()
        qa_pool.release()
        xq_pool.release()
        xt_pool.release()

        # ---------------- ctx phase (wo streams in behind it) ----------------
        ctx_pool = tc.alloc_tile_pool(name="ctx", bufs=ET)
        ctx_tiles = [ctx_pool.tile([P, SQ], G3DT, name=f"ctx{i}", tag="ctx")
                     for i in range(ET)]
        wo_pool = tc.alloc_tile_pool(name="wo", bufs=1)
        wo_full = wo_pool.tile([P, ET, D], G3DT, name="wo", tag="wo")
        for e in range(ET):
            nc.sync.dma_start(out=wo_full[:, e, :], in_=woTt[e])
        bo_bc = singles.tile([P, D], F32, name="bo_bc", tag="bo_bc")
        nc.gpsimd.dma_start(out=bo_bc, in_=bcast_ap(bod))

        for q in range(SQB):
            for e in range(ET):
                pc = psum.tile([P, SQW], F32, name="mm", tag="mm")
                for sk in range(SKT):
                    nc.tensor.matmul(
                        pc,
                        lhsT=v_tiles[sk][:, e * P:(e + 1) * P],
                        rhs=u_tiles[q][sk],
                        start=(sk == 0), stop=(sk == SKT - 1),
                    )
                nc.scalar.copy(ctx_tiles[e][:, q * SQW:(q + 1) * SQW], pc)
        u_pool.release()
        v_pool.release()

        # ---------------- out projection ----------------
        with tc.tile_pool(name="ofly", bufs=3) as o_pool:
            for st in range(SQT):
                for fb in range(NB):
                    po = psum.tile([P, NBW], F32, name="mm", tag="mm")
                    for e in range(ET):
                        nc.tensor.matmul(
                            po,
                            lhsT=ctx_tiles[e][:, st * P:(st + 1) * P],
                            rhs=wo_full[:, e, fb * NBW:(fb + 1) * NBW],
                            start=(e == 0), stop=(e == ET - 1),
                        )
                    osb = o_pool.tile([P, NBW], F32, name="osb", tag="ofly")
                    nc.vector.scalar_tensor_tensor(
                        out=osb, in0=po, scalar=rzt[:, st:st + 1],
                        in1=bo_bc[:, fb * NBW:(fb + 1) * NBW],
                        op0=ALU.mult, op1=ALU.add,
                    )
                    nc.scalar.dma_start(out=outd[st, fb], in_=osb)
        wo_pool.release()
        ctx_pool.release()

    nc.compile()
    return nc


_NC_CACHE = {}


def _get_nc():
    if "nc" not in _NC_CACHE:
        _NC_CACHE["nc"] = build_nc()
    return _NC_CACHE["nc"]


def _round_f32r(a):
    """Round-to-nearest to fp32r precision (fp22 = s1e8m13)."""
    u = np.ascontiguousarray(a, np.float32).view(np.uint32)
    u = ((u.astype(np.uint64) + 0x200) & 0xFFFFFC00).astype(np.uint32)
    return u.view(np.float32)


def _cast(a, dt):
    a = np.ascontiguousarray(np.asarray(a, np.float32))
    if dt == BF16:
        return a.astype(ml_dtypes.bfloat16)
    if dt == F32R:
        return _round_f32r(a)
    return a


def _tile_rows(m, dt):
    """[D, N] -> contiguous [D//128, 128, N] row-tiles, cast to dt."""
    m = np.asarray(m, np.float32)
    return np.ascontiguousarray(_cast(m, dt).reshape(m.shape[0] // 128, 128, -1))


def make_in_maps(x, Wq, bq, Wk, bk, Wv, bv, Wo, bo):
    x = np.asarray(x, np.float32)
    Wq = np.asarray(Wq, np.float32)
    Wk = np.asarray(Wk, np.float32)
    # A = Wq^T Wk so scores = x A x^T (+ per-key offset from bq, see header)
    aMt = _tile_rows(Wq.T @ Wk, G1DT)
    wvTt = _tile_rows(np.asarray(Wv, np.float32).T, G1DT)
    woTt = _tile_rows(np.asarray(Wo, np.float32).T, G3DT)
    bv = np.ascontiguousarray(np.asarray(bv, np.float32))
    bo = np.ascontiguousarray(np.asarray(bo, np.float32))
    ck = np.asarray(bq, np.float32) @ Wk  # [d]

    in_maps = []
    for c in range(N_CORES):
        b, h = c // 2, c % 2
        xb = x[b]  # [S, D], global key order
        own = xb[h * SQ:(h + 1) * SQ]
        xTt_c = _tile_rows(xb.T, G1DT)
        xqt_c = _tile_rows(own.T, G1DT)
        soff = np.ascontiguousarray((xb @ ck) * np.float32(SCALE))
        in_maps.append({
            "xTt": xTt_c, "xqt": xqt_c, "aMt": aMt, "wvTt": wvTt, "woTt": woTt,
            "bv": bv, "bo": bo, "soff": soff,
        })
    return in_maps


def assemble(results):
    out = np.empty((B, S, D), np.float32)
    for c in range(N_CORES):
        b, h = c // 2, c % 2
        # [8, 2, 128, 512] tiled -> [1024, 1024]
        blk = np.asarray(results[c]["out"])
        out[b, h * SQ:(h + 1) * SQ] = (
            blk.transpose(0, 2, 1, 3).reshape(SQ, D))
    return out


def kernel(x, Wq, bq, Wk, bk, Wv, bv, Wo, bo, **kwargs):
    nc = _get_nc()
    in_maps = make_in_maps(x, Wq, bq, Wk, bk, Wv, bv, Wo, bo)
    res = bass_utils.run_bass_kernel_spmd(nc, in_maps, core_ids=list(range(N_CORES)))
    return assemble(res.results)


# revision 28
# speedup vs baseline: 1.1311x; 1.0132x over previous
"""Single-head attention (nn_MultiHeadAttention) Trainium2 Bass kernel, v5.

Full inputs: x [4, 2048, 1024], Wq/Wk/Wv/Wo [1024, 1024], biases [1024].
reference:  q = x @ Wq.T + bq ; k,v likewise
            scores = (q @ k.T) / sqrt(1024) ; attn = softmax(scores, -1)
            out = (attn @ v) @ Wo.T + bo

Sharding: 8 cores = 4 batches x 2 query-halves; each core owns 1024
queries and all 2048 keys of its batch (global key order everywhere).

Algebraic fusion: scores = x (Wq^T Wk) x^T + (bq Wk) x^T + per-query
consts (which cancel in softmax).  A = Wq^T Wk is precomputed on the
host, so the K projection (and its DRAM spill) disappears: scores
contract directly against the resident x tiles.  The per-key offset
o_k = x_k . (bq Wk) rides in through the exp's per-partition bias.

V dedup: each core projects V only for its OWN 1024 keys (which equal
its own query rows, passed as the separate xq input so the program
stays SPMD-uniform), then the core pair exchanges halves with a
pairwise AllGather through a DRAM bounce, hidden behind the scores
phase.

Per-core pipeline (all matmul operands bf16, fp32 PSUM accumulation):
  QA phase:  QAT[d',q]  = A^T xq^T          (d-outer for fast start)
  V phase:   Vown[s,e]  = xq^T Wv^T + bv    (own 1024 keys only)
             spill -> AllGather[pair] -> reload full V   (async)
  scores:    u[k,q]     = exp((QAT^T x)^T * scale + o_k * scale)
             Z[q]       = sum_k u           (vector-engine accumulation)
  ctx:       ctxT[e,q]  = V^T u
  out:       out[q,f]   = (ctxT^T Wo^T) * (1/Z) + bo
"""

import numpy as np
from contextlib import ExitStack

import ml_dtypes

import concourse.bass as bass
import concourse.bacc as bacc
import concourse.mybir as mybir
import concourse.tile as tile
from concourse import bass_utils

F32 = mybir.dt.float32
F32R = mybir.dt.float32r
BF16 = mybir.dt.bfloat16
AF = mybir.ActivationFunctionType
ALU = mybir.AluOpType

B, S, D = 4, 2048, 1024
SQ = S // 2  # queries per core
N_CORES = 8
SCALE = 1.0 / float(np.sqrt(D))

# matmul operand dtypes (PSUM accumulation is always fp32)
G1DT = BF16   # x, A, qa, wv  (QA / scores / V matmuls)
G2DT = BF16   # v, u          (ctx matmuls)
G3DT = BF16   # ctx, wo       (out-projection matmuls)


def build_nc():
    P = 128
    DT = D // P          # contraction tiles (8)
    ET = D // P          # output-dim tiles (8)
    SQW = 512            # query free-dim block
    SQB = SQ // SQW      # (2)
    SQT = SQ // P        # query tiles (8)
    SKT = S // P         # key tiles (16)
    SOT = SQ // P        # own-key tiles (8)
    NBW = 512            # free-dim block over D for V/out phases
    NB = D // NBW        # (2)

    nc = bacc.Bacc("TRN2", target_bir_lowering=False, debug=False,
                   num_devices=N_CORES)

    # all inputs pre-tiled on the host so every DMA chunk is one contiguous
    # DRAM run (strided row-chunks cap DMA throughput on descriptor overhead)
    xTt = nc.dram_tensor("xTt", [DT, P, S], G1DT, kind="ExternalInput")
    xqt = nc.dram_tensor("xqt", [DT, P, SQ], G1DT, kind="ExternalInput")
    aMt = nc.dram_tensor("aMt", [DT, P, D], G1DT, kind="ExternalInput")
    wvTt = nc.dram_tensor("wvTt", [DT, P, D], G1DT, kind="ExternalInput")
    woTt = nc.dram_tensor("woTt", [ET, P, D], G3DT, kind="ExternalInput")
    bvd = nc.dram_tensor("bv", [D], F32, kind="ExternalInput")
    bod = nc.dram_tensor("bo", [D], F32, kind="ExternalInput")
    soffd = nc.dram_tensor("soff", [S], F32, kind="ExternalInput")
    outd = nc.dram_tensor("out", [SQT, NB, P, NBW], F32, kind="ExternalOutput")

    def bcast_ap(handle):
        a = handle[:]
        return bass.AP(tensor=a.tensor, offset=a.offset, ap=[[0, P]] + list(a.ap))

    with tile.TileContext(nc) as tc, ExitStack() as top:
        psum = top.enter_context(tc.tile_pool(name="psum", bufs=8, space="PSUM"))
        dram = top.enter_context(tc.tile_pool(name="dram", bufs=1, space="DRAM"))
        singles = top.enter_context(tc.tile_pool(name="singles", bufs=1))
        zscr = dram.tile([SQ], F32, name="zscr", tag="zscr")
        vb_in = dram.tile([SOT, P, D], G2DT, name="vb_in", tag="vb_in")
        vb_out = dram.tile([2, SOT, P, D], G2DT, name="vb_out", tag="vb_out")

        # ---- right-side pools, reserved in release order (LIFO top last)
        v_pool = tc.alloc_tile_pool(name="v", bufs=SKT, side="right")
        v_tiles = [v_pool.tile([P, D], G2DT, name=f"v{i}", tag="v")
                   for i in range(SKT)]
        u_pool = tc.alloc_tile_pool(name="u", bufs=SKT * SQB, side="right")
        u_tiles = [[None] * SKT for _ in range(SQB)]
        zacc_pool = tc.alloc_tile_pool(name="zacc", bufs=SQB, side="right")
        wv_pool = tc.alloc_tile_pool(name="wv", bufs=1, side="right")
        vown_pool = tc.alloc_tile_pool(name="vown", bufs=SOT, side="right")

        # ---- left-side: xt/xq under qa under a_row (released in reverse)
        xt_pool = tc.alloc_tile_pool(name="xt", bufs=DT)
        xq_pool = tc.alloc_tile_pool(name="xq", bufs=DT)
        qa_pool = tc.alloc_tile_pool(name="qa", bufs=ET)
        qa_tiles = [qa_pool.tile([P, SQ], G1DT, name=f"qa{i}", tag="qa")
                    for i in range(ET)]
        a_pool = tc.alloc_tile_pool(name="arow", bufs=DT)

        # DMA plan: scalar stays free for ACT work (PSUM evacuation); sync
        # carries xq -> xt -> V spills -> V reloads -> wo; gpsimd carries
        # biases -> wv -> a -> the AllGather -> z round-trip.  Loads are
        # emitted in consumption order: the V d-loop of sgroup 0 needs
        # wv[d] + xq[d][:, 0:512] per d-step.
        wv_full = wv_pool.tile([P, DT, D], G1DT, name="wv", tag="wv")
        xq_tiles = []
        for d in range(DT):
            xq_t = xq_pool.tile([P, SQ], G1DT, name=f"xq{d}", tag="xq")
            nc.sync.dma_start(out=xq_t, in_=xqt[d])
            nc.gpsimd.dma_start(out=wv_full[:, d, :], in_=wvTt[d])
            xq_tiles.append(xq_t)
        bv_bc = singles.tile([P, D], F32, name="bv_bc", tag="bv_bc")
        nc.gpsimd.dma_start(out=bv_bc, in_=bcast_ap(bvd))
        a_rows = []
        for d in range(DT):
            ar = a_pool.tile([P, D], G1DT, name=f"ar{d}", tag="ar")
            nc.gpsimd.dma_start(out=ar, in_=aMt[d])
            a_rows.append(ar)
        xt_tiles = []
        for t in range(DT):
            xt_t = xt_pool.tile([P, S], G1DT, name=f"xt{t}", tag="xt")
            nc.sync.dma_start(out=xt_t, in_=xTt[t])
            xt_tiles.append(xt_t)

        def xt_slice(d, lo, width):
            return xt_tiles[d][:, lo:lo + width]

        # constants / biases (emitted after the start-critical loads)
        ones_f32 = singles.tile([P, 1], F32, name="ones_f32", tag="ones_f32")
        nc.vector.memset(ones_f32, 1.0)
        ones_col = singles.tile([P, 1], F32R, name="ones_col", tag="ones_col")
        nc.scalar.activation(out=ones_col, in_=ones_f32, func=AF.Copy)
        soff_pt = singles.tile([P, SKT], F32, name="soff_pt", tag="soff_pt")
        nc.gpsimd.dma_start(out=soff_pt, in_=soffd[:].rearrange("(t p) -> p t", p=P))
        rzt = singles.tile([P, SQT], F32, name="rzt", tag="rzt")
        zt = singles.tile([P, SQT], F32, name="zt", tag="zt")
        z_sb = singles.tile([1, SQ], F32, name="z_sb", tag="z_sb")

        # ---------------- V phase first (own keys only, d-outer) ----------
        # so the pair exchange starts early and hides behind QA + scores
        vown_tiles = [vown_pool.tile([P, D], G2DT, name=f"vo{i}", tag="vo")
                      for i in range(SOT)]
        for sg in range(2):
            pv = [psum.tile([P, NBW], F32, name="mm", tag="mm") for _ in range(8)]
            for d in range(DT):
                for si in range(4):
                    for eb in range(NB):
                        nc.tensor.matmul(
                            pv[si * 2 + eb],
                            lhsT=xq_tiles[d][:, (sg * 4 + si) * P:(sg * 4 + si + 1) * P],
                            rhs=wv_full[:, d, eb * NBW:(eb + 1) * NBW],
                            start=(d == 0), stop=(d == DT - 1),
                        )
            for si in range(4):
                for eb in range(NB):
                    s = sg * 4 + si
                    nc.vector.scalar_tensor_tensor(
                        out=vown_tiles[s][:, eb * NBW:(eb + 1) * NBW],
                        in0=pv[si * 2 + eb], scalar=1.0,
                        in1=bv_bc[:, eb * NBW:(eb + 1) * NBW],
                        op0=ALU.mult, op1=ALU.add,
                    )
            for si in range(4):
                s = sg * 4 + si
                nc.sync.dma_start(out=vb_in[s], in_=vown_tiles[s])
        # pairwise exchange: AllGather the spilled halves, reload both.
        nc.gpsimd.collective_compute(
            "AllGather",
            ALU.bypass,
            replica_groups=[[0, 1], [2, 3], [4, 5], [6, 7]],
            ins=[vb_in[:]],
            outs=[vb_out[:]],
        )
        for sk in range(SKT):
            nc.gpsimd.dma_start(out=v_tiles[sk], in_=vb_out[sk // SOT, sk % SOT])

        # ---------------- QA phase (d-outer) ----------------
        for sb in range(SQB):
            pq = [psum.tile([P, SQW], F32, name="mm", tag="mm") for _ in range(ET)]
            for d in range(DT):
                for et in range(ET):
                    nc.tensor.matmul(
                        pq[et],
                        lhsT=a_rows[d][:, et * P:(et + 1) * P],
                        rhs=xq_tiles[d][:, sb * SQW:(sb + 1) * SQW],
                        start=(d == 0), stop=(d == DT - 1),
                    )
            for et in range(ET):
                nc.scalar.activation(
                    out=qa_tiles[et][:, sb * SQW:(sb + 1) * SQW],
                    in_=pq[et], func=AF.Copy,
                )
        a_pool.release()

        # ---------------- scores + Z ----------------
        for sk in range(SKT):
            for q in range(SQB):
                ps = psum.tile([P, SQW], F32, name="mm", tag="mm")
                for e in range(ET):
                    nc.tensor.matmul(
                        ps,
                        lhsT=xt_slice(e, sk * P, P),
                        rhs=qa_tiles[e][:, q * SQW:(q + 1) * SQW],
                        start=(e == 0), stop=(e == ET - 1),
                    )
                ut = u_pool.tile([P, SQW], G2DT, name=f"u{q}_{sk}", tag="u")
                nc.scalar.activation(
                    out=ut, in_=ps, func=AF.Exp,
                    bias=soff_pt[:, sk:sk + 1], scale=SCALE,
                )
                u_tiles[q][sk] = ut
                if sk == 0:
                    za = zacc_pool.tile([P, SQW], F32R, name=f"za{q}", tag="za")
                    nc.vector.tensor_copy(za, ut)
                    if q == 0:
                        zacc = [za]
                    else:
                        zacc.append(za)
                else:
                    nc.vector.tensor_tensor(
                        out=zacc[q], in0=zacc[q], in1=ut, op=ALU.add)

        vown_pool.release()
        wv_pool.release()
        qa_pool.release()
        xq_pool.release()
        xt_pool.release()

        # ---------------- ctx phase (wo streams in behind it) ----------------
        ctx_pool = tc.alloc_tile_pool(name="ctx", bufs=ET)
        ctx_tiles = [ctx_pool.tile([P, SQ], G3DT, name=f"ctx{i}", tag="ctx")
                     for i in range(ET)]
        wo_pool = tc.alloc_tile_pool(name="wo", bufs=1)
        wo_full = wo_pool.tile([P, ET, D], G3DT, name="wo", tag="wo")
        for e in range(ET):
            nc.sync.dma_start(out=wo_full[:, e, :], in_=woTt[e])
        bo_bc = singles.tile([P, D], F32, name="bo_bc", tag="bo_bc")
        nc.gpsimd.dma_start(out=bo_bc, in_=bcast_ap(bod))

        for q in range(SQB):
            for e in range(ET):
                pc = psum.tile([P, SQW], F32, name="mm", tag="mm")
                for sk in range(SKT):
                    nc.tensor.matmul(
                        pc,
                        lhsT=v_tiles[sk][:, e * P:(e + 1) * P],
                        rhs=u_tiles[q][sk],
                        start=(sk == 0), stop=(sk == SKT - 1),
                    )
                nc.scalar.copy(ctx_tiles[e][:, q * SQW:(q + 1) * SQW], pc)
            if q == 0:
                # Z -> 1/Z in [q_p, st] layout (DRAM round-trip transpose);
                # emitted mid-ctx so the zacc-chain wait never gates the PE
                # stream and the z path completes before the out phase
                for zq in range(SQB):
                    pz = psum.tile([1, SQW], F32, name="mm", tag="mm")
                    nc.tensor.matmul(pz, lhsT=ones_col, rhs=zacc[zq],
                                     start=True, stop=True)
                    nc.scalar.copy(z_sb[0:1, zq * SQW:(zq + 1) * SQW], pz)
                nc.gpsimd.dma_start(out=zscr, in_=z_sb[0:1, :])
                nc.gpsimd.dma_start(out=zt, in_=zscr[:].rearrange("(t p) -> p t", p=P))
                nc.vector.reciprocal(out=rzt, in_=zt)

        zacc_pool.release()
        u_pool.release()
        v_pool.release()

        # ---------------- out projection ----------------
        with tc.tile_pool(name="ofly", bufs=3) as o_pool:
            for st in range(SQT):
                for fb in range(NB):
                    po = psum.tile([P, NBW], F32, name="mm", tag="mm")
                    for e in range(ET):
                        nc.tensor.matmul(
                            po,
                            lhsT=ctx_tiles[e][:, st * P:(st + 1) * P],
                            rhs=wo_full[:, e, fb * NBW:(fb + 1) * NBW],
                            start=(e == 0), stop=(e == ET - 1),
                        )
                    osb = o_pool.tile([P, NBW], F32, name="osb", tag="ofly")
                    last = (st == SQT - 1 and fb == NB - 1)
                    # split the final block so its store drains sooner
                    hs = [(0, NBW // 2), (NBW // 2, NBW)] if last else [(0, NBW)]
                    for lo, hi in hs:
                        nc.vector.scalar_tensor_tensor(
                            out=osb[:, lo:hi], in0=po[:, lo:hi],
                            scalar=rzt[:, st:st + 1],
                            in1=bo_bc[:, fb * NBW + lo:fb * NBW + hi],
                            op0=ALU.mult, op1=ALU.add,
                        )
                        nc.scalar.dma_start(
                            out=outd[st, fb, :, lo:hi], in_=osb[:, lo:hi])
        wo_pool.release()
        ctx_pool.release()

    nc.compile()
    return nc


_NC_CACHE = {}


def _get_nc():
    if "nc" not in _NC_CACHE:
        _NC_CACHE["nc"] = build_nc()
    return _NC_CACHE["nc"]


def _round_f32r(a):
    """Round-to-nearest to fp32r precision (fp22 = s1e8m13)."""
    u = np.ascontiguousarray(a, np.float32).view(np.uint32)
    u = ((u.astype(np.uint64) + 0x200) & 0xFFFFFC00).astype(np.uint32)
    return u.view(np.float32)


def _cast(a, dt):
    a = np.ascontiguousarray(np.asarray(a, np.float32))
    if dt == BF16:
        return a.astype(ml_dtypes.bfloat16)
    if dt == F32R:
        return _round_f32r(a)
    return a


def _tile_rows(m, dt):
    """[D, N] -> contiguous [D//128, 128, N] row-tiles, cast to dt."""
    m = np.asarray(m, np.float32)
    return np.ascontiguousarray(_cast(m, dt).reshape(m.shape[0] // 128, 128, -1))


def make_in_maps(x, Wq, bq, Wk, bk, Wv, bv, Wo, bo):
    x = np.asarray(x, np.float32)
    Wq = np.asarray(Wq, np.float32)
    Wk = np.asarray(Wk, np.float32)
    # A = Wq^T Wk so scores = x A x^T (+ per-key offset from bq, see header)
    aMt = _tile_rows(Wq.T @ Wk, G1DT)
    wvTt = _tile_rows(np.asarray(Wv, np.float32).T, G1DT)
    woTt = _tile_rows(np.asarray(Wo, np.float32).T, G3DT)
    bv = np.ascontiguousarray(np.asarray(bv, np.float32))
    bo = np.ascontiguousarray(np.asarray(bo, np.float32))
    ck = np.asarray(bq, np.float32) @ Wk  # [d]

    in_maps = []
    for c in range(N_CORES):
        b, h = c // 2, c % 2
        xb = x[b]  # [S, D], global key order
        own = xb[h * SQ:(h + 1) * SQ]
        xTt_c = _tile_rows(xb.T, G1DT)
        xqt_c = _tile_rows(own.T, G1DT)
        soff = np.ascontiguousarray((xb @ ck) * np.float32(SCALE))
        in_maps.append({
            "xTt": xTt_c, "xqt": xqt_c, "aMt": aMt, "wvTt": wvTt, "woTt": woTt,
            "bv": bv, "bo": bo, "soff": soff,
        })
    return in_maps


def assemble(results):
    out = np.empty((B, S, D), np.float32)
    for c in range(N_CORES):
        b, h = c // 2, c % 2
        # [8, 2, 128, 512] tiled -> [1024, 1024]
        blk = np.asarray(results[c]["out"])
        out[b, h * SQ:(h + 1) * SQ] = (
            blk.transpose(0, 2, 1, 3).reshape(SQ, D))
    return out


def kernel(x, Wq, bq, Wk, bk, Wv, bv, Wo, bo, **kwargs):
    nc = _get_nc()
    in_maps = make_in_maps(x, Wq, bq, Wk, bk, Wv, bv, Wo, bo)
    res = bass_utils.run_bass_kernel_spmd(nc, in_maps, core_ids=list(range(N_CORES)))
    return assemble(res.results)


# revision 37
# speedup vs baseline: 1.3011x; 1.1503x over previous
"""Single-head attention (nn_MultiHeadAttention) Trainium2 Bass kernel, v8.

Full inputs: x [4, 2048, 1024], Wq/Wk/Wv/Wo [1024, 1024], biases [1024].
reference:  q = x @ Wq.T + bq ; k,v likewise
            scores = (q @ k.T) / sqrt(1024) ; attn = softmax(scores, -1)
            out = (attn @ v) @ Wo.T + bo

Sharding: 8 cores = 4 batches x 2 query-halves; each core owns 1024
queries and all 2048 keys of its batch (global key order everywhere).

Algebraic fusions (host-side weight transforms):
  scores:  q k^T = x (Wq^T Wk) x^T + (bq Wk) x^T + per-query consts that
           cancel in softmax.  A = Wq^T Wk is precomputed on the host, so
           the K projection disappears; the per-key offset o_k = x_k.(bq Wk)
           rides in through the exp's per-partition bias.
  output:  (attn @ (x Wv^T + bv)) Wo^T + bo = attn @ (x (Wo Wv)^T + bc)
           with bc = Wo bv + bo, because the softmax rows sum to 1.  With
           Wvo = Wo Wv precomputed on the host, the ctx matmul yields the
           FINAL output directly — no separate out-projection phase.

V dedup: each core projects VO' = x (Wo Wv)^T + bc only for its OWN 1024
keys (which equal its own query rows, passed as the separate xq input so
the program stays SPMD-uniform), then the core pair exchanges halves via
a pairwise AllGather through a DRAM bounce, hidden behind QA + scores.

Per-core pipeline (all matmul operands bf16, fp32 PSUM accumulation):
  VO phase:  VO'[s,f]  = xq^T Wvo^T + bc     (own keys, d-outer, first)
             spill -> AllGather[pair] -> reload full VO'   (async)
  QA phase:  QAT[d',q] = A^T xq^T            (d-outer)
  scores:    u[k,q]    = exp((QAT^T x)^T * scale + o_k * scale)
             Z[q]      = sum_k u             (vector-engine accumulation)
  out:       out[f,q]  = (VO'^T u) * (1/Z)   (stored f-major, host untiles)
"""

import numpy as np
from contextlib import ExitStack

import ml_dtypes

import concourse.bass as bass
import concourse.bacc as bacc
import concourse.bass_isa as bass_isa
import concourse.mybir as mybir
import concourse.tile as tile
from concourse import bass_utils

F32 = mybir.dt.float32
F32R = mybir.dt.float32r
BF16 = mybir.dt.bfloat16
AF = mybir.ActivationFunctionType
ALU = mybir.AluOpType

B, S, D = 4, 2048, 1024
SQ = S // 2  # queries per core
N_CORES = 8
SCALE = 1.0 / float(np.sqrt(D))

# matmul operand dtypes (PSUM accumulation is always fp32)
G1DT = BF16   # x, A, qa, wvo  (QA / scores / VO matmuls)
G2DT = BF16   # vo, u          (ctx matmuls)


def build_nc():
    P = 128
    DT = D // P          # contraction tiles (8)
    ET = D // P          # output-dim tiles (8)
    SQW = 512            # query free-dim block
    SQB = SQ // SQW      # (2)
    SKT = S // P         # key tiles (16)
    SOT = SQ // P        # own-key tiles (8)
    NBW = 512            # free-dim block over D for the VO phase
    NB = D // NBW        # (2)

    nc = bacc.Bacc("TRN2", target_bir_lowering=False, debug=False,
                   num_devices=N_CORES)

    # all inputs pre-tiled on the host so every DMA chunk is one contiguous
    # DRAM run (strided row-chunks cap DMA throughput on descriptor overhead)
    xTt = nc.dram_tensor("xTt", [DT, P, S], G1DT, kind="ExternalInput")
    xqt = nc.dram_tensor("xqt", [DT, P, SQ], G1DT, kind="ExternalInput")
    aMt = nc.dram_tensor("aMt", [DT, P, D], G1DT, kind="ExternalInput")
    wvoTt = nc.dram_tensor("wvoTt", [DT, P, D], G1DT, kind="ExternalInput")
    bcd = nc.dram_tensor("bc", [D], F32, kind="ExternalInput")
    soffd = nc.dram_tensor("soff", [S], F32, kind="ExternalInput")
    outd = nc.dram_tensor("out", [ET, SQB, P, SQW], F32, kind="ExternalOutput")

    def bcast_ap(handle):
        a = handle[:]
        return bass.AP(tensor=a.tensor, offset=a.offset, ap=[[0, P]] + list(a.ap))

    with tile.TileContext(nc) as tc, ExitStack() as top:
        psum = top.enter_context(tc.tile_pool(name="psum", bufs=8, space="PSUM"))
        dram = top.enter_context(tc.tile_pool(name="dram", bufs=1, space="DRAM"))
        singles = top.enter_context(tc.tile_pool(name="singles", bufs=1))
        vb_in = dram.tile([SOT, P, D], G2DT, name="vb_in", tag="vb_in")
        vb_out = dram.tile([2, SOT, P, D], G2DT, name="vb_out", tag="vb_out")

        # ---- right-side pools, reserved in release order (LIFO top last)
        v_pool = tc.alloc_tile_pool(name="v", bufs=SKT, side="right")
        v_tiles = [v_pool.tile([P, D], G2DT, name=f"v{i}", tag="v")
                   for i in range(SKT)]
        u_pool = tc.alloc_tile_pool(name="u", bufs=SKT * SQB, side="right")
        u_tiles = [[None] * SKT for _ in range(SQB)]
        zacc_pool = tc.alloc_tile_pool(name="zacc", bufs=SQB, side="right")
        wv_pool = tc.alloc_tile_pool(name="wv", bufs=1, side="right")
        vown_pool = tc.alloc_tile_pool(name="vown", bufs=SOT, side="right")

        # ---- left-side: xt/xq under qa under a_row (released in reverse)
        xt_pool = tc.alloc_tile_pool(name="xt", bufs=DT)
        xq_pool = tc.alloc_tile_pool(name="xq", bufs=DT)
        qa_pool = tc.alloc_tile_pool(name="qa", bufs=ET)
        qa_tiles = [qa_pool.tile([P, SQ], G1DT, name=f"qa{i}", tag="qa")
                    for i in range(ET)]
        a_pool = tc.alloc_tile_pool(name="arow", bufs=DT)

        # DMA plan: scalar stays free for ACT work (PSUM evacuation); sync
        # carries xq -> xt -> VO spills; gpsimd carries wvo -> bc -> a ->
        # the AllGather -> VO reloads -> z round-trip.  Loads are emitted in
        # consumption order: the VO d-loop of sgroup 0 needs wvo[d] +
        # xq[d][:, 0:512] per d-step.
        wv_full = wv_pool.tile([P, DT, D], G1DT, name="wv", tag="wv")
        xq_tiles = []
        for d in range(DT):
            xq_t = xq_pool.tile([P, SQ], G1DT, name=f"xq{d}", tag="xq")
            nc.sync.dma_start(out=xq_t, in_=xqt[d])
            nc.gpsimd.dma_start(out=wv_full[:, d, :], in_=wvoTt[d])
            xq_tiles.append(xq_t)
        bc_bc = singles.tile([P, D], F32, name="bc_bc", tag="bc_bc")
        nc.gpsimd.dma_start(out=bc_bc, in_=bcast_ap(bcd))
        a_rows = []
        for d in range(DT):
            ar = a_pool.tile([P, D], G1DT, name=f"ar{d}", tag="ar")
            nc.gpsimd.dma_start(out=ar, in_=aMt[d])
            a_rows.append(ar)
        xt_tiles = []
        for t in range(DT):
            xt_t = xt_pool.tile([P, S], G1DT, name=f"xt{t}", tag="xt")
            nc.sync.dma_start(out=xt_t, in_=xTt[t])
            xt_tiles.append(xt_t)

        def xt_slice(d, lo, width):
            return xt_tiles[d][:, lo:lo + width]

        # constants (emitted after the start-critical loads)
        soff_pt = singles.tile([P, SKT], F32, name="soff_pt", tag="soff_pt")
        nc.gpsimd.dma_start(out=soff_pt, in_=soffd[:].rearrange("(t p) -> p t", p=P))
        rz_bc = singles.tile([P, SQ], F32, name="rz_bc", tag="rz_bc")

        # ---------------- VO phase first (own keys only, d-outer) ----------
        # so the pair exchange starts early and hides behind QA + scores
        vown_tiles = [vown_pool.tile([P, D], G2DT, name=f"vo{i}", tag="vo")
                      for i in range(SOT)]
        for sg in range(2):
            pv = [psum.tile([P, NBW], F32, name="mm", tag="mm") for _ in range(8)]
            for d in range(DT):
                for si in range(4):
                    for eb in range(NB):
                        nc.tensor.matmul(
                            pv[si * 2 + eb],
                            lhsT=xq_tiles[d][:, (sg * 4 + si) * P:(sg * 4 + si + 1) * P],
                            rhs=wv_full[:, d, eb * NBW:(eb + 1) * NBW],
                            start=(d == 0), stop=(d == DT - 1),
                        )
            for si in range(4):
                for eb in range(NB):
                    s = sg * 4 + si
                    nc.vector.scalar_tensor_tensor(
                        out=vown_tiles[s][:, eb * NBW:(eb + 1) * NBW],
                        in0=pv[si * 2 + eb], scalar=1.0,
                        in1=bc_bc[:, eb * NBW:(eb + 1) * NBW],
                        op0=ALU.mult, op1=ALU.add,
                    )
            for si in range(4):
                s = sg * 4 + si
                nc.sync.dma_start(out=vb_in[s], in_=vown_tiles[s])
        # pairwise exchange: AllGather the spilled halves, reload both.
        nc.gpsimd.collective_compute(
            "AllGather",
            ALU.bypass,
            replica_groups=[[0, 1], [2, 3], [4, 5], [6, 7]],
            ins=[vb_in[:]],
            outs=[vb_out[:]],
        )
        for sk in range(SKT):
            nc.gpsimd.dma_start(out=v_tiles[sk], in_=vb_out[sk // SOT, sk % SOT])

        # ---------------- QA phase (d-outer) ----------------
        for sb in range(SQB):
            pq = [psum.tile([P, SQW], F32, name="mm", tag="mm") for _ in range(ET)]
            for d in range(DT):
                for et in range(ET):
                    nc.tensor.matmul(
                        pq[et],
                        lhsT=a_rows[d][:, et * P:(et + 1) * P],
                        rhs=xq_tiles[d][:, sb * SQW:(sb + 1) * SQW],
                        start=(d == 0), stop=(d == DT - 1),
                    )
            for et in range(ET):
                nc.scalar.activation(
                    out=qa_tiles[et][:, sb * SQW:(sb + 1) * SQW],
                    in_=pq[et], func=AF.Copy,
                )
        a_pool.release()

        # ---------------- scores + Z ----------------
        for sk in range(SKT):
            for q in range(SQB):
                ps = psum.tile([P, SQW], F32, name="mm", tag="mm")
                for e in range(ET):
                    nc.tensor.matmul(
                        ps,
                        lhsT=xt_slice(e, sk * P, P),
                        rhs=qa_tiles[e][:, q * SQW:(q + 1) * SQW],
                        start=(e == 0), stop=(e == ET - 1),
                    )
                ut = u_pool.tile([P, SQW], G2DT, name=f"u{q}_{sk}", tag="u")
                nc.scalar.activation(
                    out=ut, in_=ps, func=AF.Exp,
                    bias=soff_pt[:, sk:sk + 1], scale=SCALE,
                )
                u_tiles[q][sk] = ut
                if sk == 0:
                    za = zacc_pool.tile([P, SQW], F32R, name=f"za{q}", tag="za")
                    nc.vector.tensor_copy(za, ut)
                    if q == 0:
                        zacc = [za]
                    else:
                        zacc.append(za)
                else:
                    nc.vector.tensor_tensor(
                        out=zacc[q], in0=zacc[q], in1=ut, op=ALU.add)

        # Z -> 1/Z replicated across partitions, entirely off the PE queue:
        # gpsimd cross-partition all-reduce, then a DVE reciprocal
        for zq in range(SQB):
            zsum = singles.tile([P, SQW], F32, name=f"zsum{zq}", tag=f"zsum{zq}")
            nc.gpsimd.partition_all_reduce(
                zsum[:], zacc[zq][:], P, bass_isa.ReduceOp.add)
            nc.vector.reciprocal(
                out=rz_bc[:, zq * SQW:(zq + 1) * SQW], in_=zsum)

        vown_pool.release()
        wv_pool.release()
        zacc_pool.release()
        qa_pool.release()
        xq_pool.release()
        xt_pool.release()

        # ---------------- fused ctx/out phase ----------------
        with tc.tile_pool(name="ofly", bufs=4) as o_pool:
            for q in range(SQB):
                for e in range(ET):
                    pc = psum.tile([P, SQW], F32, name="mm", tag="mm")
                    for sk in range(SKT):
                        nc.tensor.matmul(
                            pc,
                            lhsT=v_tiles[sk][:, e * P:(e + 1) * P],
                            rhs=u_tiles[q][sk],
                            start=(sk == 0), stop=(sk == SKT - 1),
                        )
                    osb = o_pool.tile([P, SQW], F32, name="osb", tag="ofly")
                    nc.vector.tensor_tensor(
                        out=osb, in0=pc,
                        in1=rz_bc[:, q * SQW:(q + 1) * SQW], op=ALU.mult)
                    nc.scalar.dma_start(out=outd[e, q], in_=osb)
        u_pool.release()
        v_pool.release()

    nc.compile()
    return nc


_NC_CACHE = {}


def _get_nc():
    if "nc" not in _NC_CACHE:
        _NC_CACHE["nc"] = build_nc()
    return _NC_CACHE["nc"]


def _round_f32r(a):
    """Round-to-nearest to fp32r precision (fp22 = s1e8m13)."""
    u = np.ascontiguousarray(a, np.float32).view(np.uint32)
    u = ((u.astype(np.uint64) + 0x200) & 0xFFFFFC00).astype(np.uint32)
    return u.view(np.float32)


def _cast(a, dt):
    a = np.ascontiguousarray(np.asarray(a, np.float32))
    if dt == BF16:
        return a.astype(ml_dtypes.bfloat16)
    if dt == F32R:
        return _round_f32r(a)
    return a


def _tile_rows(m, dt):
    """[D, N] -> contiguous [D//128, 128, N] row-tiles, cast to dt."""
    m = np.asarray(m, np.float32)
    return np.ascontiguousarray(_cast(m, dt).reshape(m.shape[0] // 128, 128, -1))


def make_in_maps(x, Wq, bq, Wk, bk, Wv, bv, Wo, bo):
    x = np.asarray(x, np.float32)
    Wq = np.asarray(Wq, np.float32)
    Wk = np.asarray(Wk, np.float32)
    Wv = np.asarray(Wv, np.float32)
    Wo = np.asarray(Wo, np.float32)
    # A = Wq^T Wk so scores = x A x^T (+ per-key offset from bq, see header)
    aMt = _tile_rows(Wq.T @ Wk, G1DT)
    # Wvo = Wo Wv folds the output projection into the value path; the
    # matching bias constant is bc = Wo bv + bo (softmax rows sum to 1)
    wvoTt = _tile_rows((Wo @ Wv).T, G1DT)
    bc = np.ascontiguousarray(Wo @ np.asarray(bv, np.float32)
                              + np.asarray(bo, np.float32))
    ck = np.asarray(bq, np.float32) @ Wk  # [d]

    in_maps = []
    for c in range(N_CORES):
        b, h = c // 2, c % 2
        xb = x[b]  # [S, D], global key order
        own = xb[h * SQ:(h + 1) * SQ]
        xTt_c = _tile_rows(xb.T, G1DT)
        xqt_c = _tile_rows(own.T, G1DT)
        soff = np.ascontiguousarray((xb @ ck) * np.float32(SCALE))
        in_maps.append({
            "xTt": xTt_c, "xqt": xqt_c, "aMt": aMt, "wvoTt": wvoTt,
            "bc": bc, "soff": soff,
        })
    return in_maps


def assemble(results):
    out = np.empty((B, S, D), np.float32)
    for c in range(N_CORES):
        b, h = c // 2, c % 2
        # [8(e), 2(qb), 128(f), 512(q)] tiled, f-major -> [1024 q, 1024 f]
        blk = np.asarray(results[c]["out"])
        out[b, h * SQ:(h + 1) * SQ] = (
            blk.transpose(1, 3, 0, 2).reshape(SQ, D))
    return out


def kernel(x, Wq, bq, Wk, bk, Wv, bv, Wo, bo, **kwargs):
    nc = _get_nc()
    in_maps = make_in_maps(x, Wq, bq, Wk, bk, Wv, bv, Wo, bo)
    res = bass_utils.run_bass_kernel_spmd(nc, in_maps, core_ids=list(range(N_CORES)))
    return assemble(res.results)


# revision 39
# speedup vs baseline: 1.3154x; 1.0110x over previous
"""Single-head attention (nn_MultiHeadAttention) Trainium2 Bass kernel.

Full inputs: x [4, 2048, 1024], Wq/Wk/Wv/Wo [1024, 1024], biases [1024].
reference:  q = x @ Wq.T + bq ; k,v likewise
            scores = (q @ k.T) / sqrt(1024) ; attn = softmax(scores, -1)
            out = (attn @ v) @ Wo.T + bo

Sharding: 8 cores = 4 batches x 2 query-halves; each core owns 1024
queries and all 2048 keys of its batch (global key order everywhere).

Algebraic fusions (host-side weight transforms):
  scores:  q k^T = x (Wq^T Wk) x^T + (bq Wk) x^T + per-query consts that
           cancel in softmax.  A = Wq^T Wk is precomputed on the host, so
           the K projection disappears; the per-key offset o_k = x_k.(bq Wk)
           rides in through the exp's per-partition bias.
  output:  (attn @ (x Wv^T + bv)) Wo^T + bo = attn @ (x (Wo Wv)^T + bc)
           with bc = Wo bv + bo, because the softmax rows sum to 1.  With
           Wvo = Wo Wv precomputed on the host, the ctx matmul yields the
           FINAL output directly — no separate out-projection phase.

V dedup: each core projects VO' = x (Wo Wv)^T + bc only for its OWN 1024
keys (which equal its own query rows, passed as the separate xq input so
the program stays SPMD-uniform), then the core pair exchanges halves via
a pairwise AllGather through a DRAM bounce, hidden behind QA + scores.

Per-core pipeline (all matmul operands bf16, fp32 PSUM accumulation):
  VO phase:  VO'[s,f]  = xq^T Wvo^T + bc     (own keys, d-outer, first)
             spill -> AllGather[pair] -> reload full VO'   (async)
  QA phase:  QAT[d',q] = A^T xq^T            (d-outer)
  scores:    u[k,q]    = exp((QAT^T x)^T * scale + o_k * scale)
             Z[q]      = sum_k u             (DVE accumulation + gpsimd
                                              cross-partition all-reduce)
  out:       out[f,q]  = (VO'^T u) * (1/Z)   (stored f-major, host untiles)
"""

import numpy as np
from contextlib import ExitStack

import ml_dtypes

import concourse.bass as bass
import concourse.bacc as bacc
import concourse.bass_isa as bass_isa
import concourse.mybir as mybir
import concourse.tile as tile
from concourse import bass_utils

F32 = mybir.dt.float32
F32R = mybir.dt.float32r
BF16 = mybir.dt.bfloat16
AF = mybir.ActivationFunctionType
ALU = mybir.AluOpType

B, S, D = 4, 2048, 1024
SQ = S // 2  # queries per core
N_CORES = 8
SCALE = 1.0 / float(np.sqrt(D))

# matmul operand dtypes (PSUM accumulation is always fp32)
G1DT = BF16   # x, A, qa, wvo  (QA / scores / VO matmuls)
G2DT = BF16   # vo, u          (ctx matmuls)


def build_nc():
    P = 128
    DT = D // P          # contraction tiles (8)
    ET = D // P          # output-dim tiles (8)
    SQW = 512            # query free-dim block
    SQB = SQ // SQW      # (2)
    SKT = S // P         # key tiles (16)
    SOT = SQ // P        # own-key tiles (8)
    NBW = 512            # free-dim block over D for the VO phase
    NB = D // NBW        # (2)

    nc = bacc.Bacc("TRN2", target_bir_lowering=False, debug=False,
                   num_devices=N_CORES)

    # all inputs pre-tiled on the host so every DMA chunk is one contiguous
    # DRAM run (strided row-chunks cap DMA throughput on descriptor overhead)
    xTt = nc.dram_tensor("xTt", [DT, P, S], G1DT, kind="ExternalInput")
    xqt = nc.dram_tensor("xqt", [DT, P, SQ], G1DT, kind="ExternalInput")
    aMt = nc.dram_tensor("aMt", [DT, P, D], G1DT, kind="ExternalInput")
    wvoTt = nc.dram_tensor("wvoTt", [DT, P, D], G1DT, kind="ExternalInput")
    bcd = nc.dram_tensor("bc", [D], F32, kind="ExternalInput")
    soffd = nc.dram_tensor("soff", [S], F32, kind="ExternalInput")
    outd = nc.dram_tensor("out", [ET, SQB, P, SQW], F32, kind="ExternalOutput")

    def bcast_ap(handle):
        a = handle[:]
        return bass.AP(tensor=a.tensor, offset=a.offset, ap=[[0, P]] + list(a.ap))

    with tile.TileContext(nc) as tc, ExitStack() as top:
        psum = top.enter_context(tc.tile_pool(name="psum", bufs=8, space="PSUM"))
        dram = top.enter_context(tc.tile_pool(name="dram", bufs=1, space="DRAM"))
        singles = top.enter_context(tc.tile_pool(name="singles", bufs=1))
        vb_in = dram.tile([SOT, P, D], G2DT, name="vb_in", tag="vb_in")
        vb_out = dram.tile([2, SOT, P, D], G2DT, name="vb_out", tag="vb_out")

        # ---- right-side pools, reserved in release order (LIFO top last)
        v_pool = tc.alloc_tile_pool(name="v", bufs=SKT, side="right")
        v_tiles = [v_pool.tile([P, D], G2DT, name=f"v{i}", tag="v")
                   for i in range(SKT)]
        u_pool = tc.alloc_tile_pool(name="u", bufs=SKT * SQB, side="right")
        u_tiles = [[None] * SKT for _ in range(SQB)]
        zacc_pool = tc.alloc_tile_pool(name="zacc", bufs=SQB, side="right")
        wv_pool = tc.alloc_tile_pool(name="wv", bufs=1, side="right")
        vown_pool = tc.alloc_tile_pool(name="vown", bufs=SOT, side="right")

        # ---- left-side: xt/xq under qa under a_row (released in reverse)
        xt_pool = tc.alloc_tile_pool(name="xt", bufs=DT)
        xq_pool = tc.alloc_tile_pool(name="xq", bufs=DT)
        qa_pool = tc.alloc_tile_pool(name="qa", bufs=ET)
        qa_tiles = [qa_pool.tile([P, SQ], G1DT, name=f"qa{i}", tag="qa")
                    for i in range(ET)]
        a_pool = tc.alloc_tile_pool(name="arow", bufs=DT)

        # DMA plan: scalar stays free for ACT work (PSUM evacuation); sync
        # carries xq -> xt -> VO spills; gpsimd carries wvo -> bc -> a ->
        # the AllGather -> VO reloads -> z round-trip.  Loads are emitted in
        # consumption order: the VO d-loop of sgroup 0 needs wvo[d] +
        # xq[d][:, 0:512] per d-step.
        wv_full = wv_pool.tile([P, DT, D], G1DT, name="wv", tag="wv")
        xq_tiles = []
        for d in range(DT):
            xq_t = xq_pool.tile([P, SQ], G1DT, name=f"xq{d}", tag="xq")
            nc.sync.dma_start(out=xq_t, in_=xqt[d])
            nc.gpsimd.dma_start(out=wv_full[:, d, :], in_=wvoTt[d])
            xq_tiles.append(xq_t)
        bc_bc = singles.tile([P, D], F32, name="bc_bc", tag="bc_bc")
        nc.gpsimd.dma_start(out=bc_bc, in_=bcast_ap(bcd))
        a_rows = []
        for d in range(DT):
            ar = a_pool.tile([P, D], G1DT, name=f"ar{d}", tag="ar")
            nc.gpsimd.dma_start(out=ar, in_=aMt[d])
            a_rows.append(ar)
        xt_tiles = []
        for t in range(DT):
            xt_t = xt_pool.tile([P, S], G1DT, name=f"xt{t}", tag="xt")
            nc.sync.dma_start(out=xt_t, in_=xTt[t])
            xt_tiles.append(xt_t)

        def xt_slice(d, lo, width):
            return xt_tiles[d][:, lo:lo + width]

        # constants (emitted after the start-critical loads)
        soff_pt = singles.tile([P, SKT], F32, name="soff_pt", tag="soff_pt")
        nc.gpsimd.dma_start(out=soff_pt, in_=soffd[:].rearrange("(t p) -> p t", p=P))
        rz_bc = singles.tile([P, SQ], F32, name="rz_bc", tag="rz_bc")

        # ---------------- VO phase first (own keys only, d-outer) ----------
        # so the pair exchange starts early and hides behind QA + scores
        vown_tiles = [vown_pool.tile([P, D], G2DT, name=f"vo{i}", tag="vo")
                      for i in range(SOT)]
        for sg in range(2):
            pv = [psum.tile([P, NBW], F32, name="mm", tag="mm") for _ in range(8)]
            for d in range(DT):
                for si in range(4):
                    for eb in range(NB):
                        nc.tensor.matmul(
                            pv[si * 2 + eb],
                            lhsT=xq_tiles[d][:, (sg * 4 + si) * P:(sg * 4 + si + 1) * P],
                            rhs=wv_full[:, d, eb * NBW:(eb + 1) * NBW],
                            start=(d == 0), stop=(d == DT - 1),
                        )
            for si in range(4):
                for eb in range(NB):
                    s = sg * 4 + si
                    nc.vector.scalar_tensor_tensor(
                        out=vown_tiles[s][:, eb * NBW:(eb + 1) * NBW],
                        in0=pv[si * 2 + eb], scalar=1.0,
                        in1=bc_bc[:, eb * NBW:(eb + 1) * NBW],
                        op0=ALU.mult, op1=ALU.add,
                    )
            for si in range(4):
                s = sg * 4 + si
                nc.sync.dma_start(out=vb_in[s], in_=vown_tiles[s])
        # pairwise exchange: AllGather the spilled halves, reload both.
        nc.gpsimd.collective_compute(
            "AllGather",
            ALU.bypass,
            replica_groups=[[0, 1], [2, 3], [4, 5], [6, 7]],
            ins=[vb_in[:]],
            outs=[vb_out[:]],
        )
        for sk in range(SKT):
            nc.gpsimd.dma_start(out=v_tiles[sk], in_=vb_out[sk // SOT, sk % SOT])

        # ---------------- QA phase (d-outer) ----------------
        for sb in range(SQB):
            pq = [psum.tile([P, SQW], F32, name="mm", tag="mm") for _ in range(ET)]
            for d in range(DT):
                for et in range(ET):
                    nc.tensor.matmul(
                        pq[et],
                        lhsT=a_rows[d][:, et * P:(et + 1) * P],
                        rhs=xq_tiles[d][:, sb * SQW:(sb + 1) * SQW],
                        start=(d == 0), stop=(d == DT - 1),
                    )
            for et in range(ET):
                nc.scalar.activation(
                    out=qa_tiles[et][:, sb * SQW:(sb + 1) * SQW],
                    in_=pq[et], func=AF.Copy,
                )
        a_pool.release()

        # ---------------- scores + Z ----------------
        for sk in range(SKT):
            for q in range(SQB):
                ps = psum.tile([P, SQW], F32, name="mm", tag="mm")
                for e in range(ET):
                    nc.tensor.matmul(
                        ps,
                        lhsT=xt_slice(e, sk * P, P),
                        rhs=qa_tiles[e][:, q * SQW:(q + 1) * SQW],
                        start=(e == 0), stop=(e == ET - 1),
                    )
                ut = u_pool.tile([P, SQW], G2DT, name=f"u{q}_{sk}", tag="u")
                nc.scalar.activation(
                    out=ut, in_=ps, func=AF.Exp,
                    bias=soff_pt[:, sk:sk + 1], scale=SCALE,
                )
                u_tiles[q][sk] = ut
                if sk == 0:
                    za = zacc_pool.tile([P, SQW], F32R, name=f"za{q}", tag="za")
                    nc.vector.tensor_copy(za, ut)
                    if q == 0:
                        zacc = [za]
                    else:
                        zacc.append(za)
                else:
                    nc.vector.tensor_tensor(
                        out=zacc[q], in0=zacc[q], in1=ut, op=ALU.add)

        # Z -> 1/Z replicated across partitions, entirely off the PE queue:
        # gpsimd cross-partition all-reduce, then a DVE reciprocal
        for zq in range(SQB):
            zsum = singles.tile([P, SQW], F32, name=f"zsum{zq}", tag=f"zsum{zq}")
            nc.gpsimd.partition_all_reduce(
                zsum[:], zacc[zq][:], P, bass_isa.ReduceOp.add)
            nc.vector.reciprocal(
                out=rz_bc[:, zq * SQW:(zq + 1) * SQW], in_=zsum)

        vown_pool.release()
        wv_pool.release()
        zacc_pool.release()
        qa_pool.release()
        xq_pool.release()
        xt_pool.release()

        # ---------------- fused ctx/out phase ----------------
        with tc.tile_pool(name="ofly", bufs=4) as o_pool:
            for q in range(SQB):
                for e in range(ET):
                    pc = psum.tile([P, SQW], F32, name="mm", tag="mm")
                    for sk in range(SKT):
                        nc.tensor.matmul(
                            pc,
                            lhsT=v_tiles[sk][:, e * P:(e + 1) * P],
                            rhs=u_tiles[q][sk],
                            start=(sk == 0), stop=(sk == SKT - 1),
                        )
                    osb = o_pool.tile([P, SQW], F32, name="osb", tag="ofly")
                    nc.vector.tensor_tensor(
                        out=osb, in0=pc,
                        in1=rz_bc[:, q * SQW:(q + 1) * SQW], op=ALU.mult)
                    nc.scalar.dma_start(out=outd[e, q], in_=osb)
        u_pool.release()
        v_pool.release()

    nc.compile()
    return nc


_NC_CACHE = {}


def _get_nc():
    if "nc" not in _NC_CACHE:
        _NC_CACHE["nc"] = build_nc()
    return _NC_CACHE["nc"]


def _round_f32r(a):
    """Round-to-nearest to fp32r precision (fp22 = s1e8m13)."""
    u = np.ascontiguousarray(a, np.float32).view(np.uint32)
    u = ((u.astype(np.uint64) + 0x200) & 0xFFFFFC00).astype(np.uint32)
    return u.view(np.float32)


def _cast(a, dt):
    a = np.ascontiguousarray(np.asarray(a, np.float32))
    if dt == BF16:
        return a.astype(ml_dtypes.bfloat16)
    if dt == F32R:
        return _round_f32r(a)
    return a


def _tile_rows(m, dt):
    """[D, N] -> contiguous [D//128, 128, N] row-tiles, cast to dt."""
    m = np.asarray(m, np.float32)
    return np.ascontiguousarray(_cast(m, dt).reshape(m.shape[0] // 128, 128, -1))


def make_in_maps(x, Wq, bq, Wk, bk, Wv, bv, Wo, bo):
    x = np.asarray(x, np.float32)
    Wq = np.asarray(Wq, np.float32)
    Wk = np.asarray(Wk, np.float32)
    Wv = np.asarray(Wv, np.float32)
    Wo = np.asarray(Wo, np.float32)
    # A = Wq^T Wk so scores = x A x^T (+ per-key offset from bq, see header)
    aMt = _tile_rows(Wq.T @ Wk, G1DT)
    # Wvo = Wo Wv folds the output projection into the value path; the
    # matching bias constant is bc = Wo bv + bo (softmax rows sum to 1)
    wvoTt = _tile_rows((Wo @ Wv).T, G1DT)
    bc = np.ascontiguousarray(Wo @ np.asarray(bv, np.float32)
                              + np.asarray(bo, np.float32))
    ck = np.asarray(bq, np.float32) @ Wk  # [d]

    in_maps = []
    for c in range(N_CORES):
        b, h = c // 2, c % 2
        xb = x[b]  # [S, D], global key order
        own = xb[h * SQ:(h + 1) * SQ]
        xTt_c = _tile_rows(xb.T, G1DT)
        xqt_c = _tile_rows(own.T, G1DT)
        soff = np.ascontiguousarray((xb @ ck) * np.float32(SCALE))
        in_maps.append({
            "xTt": xTt_c, "xqt": xqt_c, "aMt": aMt, "wvoTt": wvoTt,
            "bc": bc, "soff": soff,
        })
    return in_maps


def assemble(results):
    out = np.empty((B, S, D), np.float32)
    for c in range(N_CORES):
        b, h = c // 2, c % 2
        # [8(e), 2(qb), 128(f), 512(q)] tiled, f-major -> [1024 q, 1024 f]
        blk = np.asarray(results[c]["out"])
        out[b, h * SQ:(h + 1) * SQ] = (
            blk.transpose(1, 3, 0, 2).reshape(SQ, D))
    return out


def kernel(x, Wq, bq, Wk, bk, Wv, bv, Wo, bo, **kwargs):
    nc = _get_nc()
    in_maps = make_in_maps(x, Wq, bq, Wk, bk, Wv, bv, Wo, bo)
    res = bass_utils.run_bass_kernel_spmd(nc, in_maps, core_ids=list(range(N_CORES)))
    return assemble(res.results)
